# revision 7
# baseline (speedup 1.0000x reference)
"""Trainium2 Bass kernel: two chained SAME-padded 3x3 single-channel convs.

  reference: z = conv3x3(conv3x3(x, w1), w2)   x: [16,1,2048,2048] f32

Strategy (pure data parallel, 2 images per core on 8 cores):
  - Images processed in horizontal bands of S=124 output rows.
  - Each conv is computed on the TensorEngine as 3 banded matmuls
    (one per kernel column dx), accumulating in PSUM. The band matrix
    B_dx[k, m] = W[k-m, dx] applies the vertical taps; the horizontal
    taps come from shifting the moving operand (rhs) by dx columns.
  - conv1 output (y) is copied PSUM->SBUF (VectorE) and consumed by
    conv2's matmuls; conv2 output is copied PSUM->SBUF (ScalarE) and
    DMA'd out. Intermediate y never touches HBM.
  - float32r matmuls (1 cycle/row on the PE vs 4 for fp32; ~1.5e-4
    absmax-relative rounding measured on HW). Set MM_DT = F32 below for
    full fp32 precision at ~4x the PE cost.
  - SAME padding handled with zeroed halo columns in SBUF and
    host-built band-matrix variants for the top/bottom image edges.

Band matrices are built on the host from w1/w2 (they are just 9 floats
each) and passed as extra inputs.
"""

import hashlib
import os
import shutil

import numpy as np

import concourse.mybir as mybir
import concourse.tile as tile
from concourse import bacc, bass2jax
from concourse.bass_utils import run_bass_kernel_spmd


def _install_neff_disk_cache():
    """Cache compiled NEFFs on disk keyed by BIR content hash — the
    neuronxcc backend takes minutes for this kernel and has no cache of
    its own, so a fresh process would otherwise recompile every run."""
    if getattr(bass2jax, "_ant_neff_cache_installed", False):
        return
    orig = bass2jax.compile_bir_kernel

    def cached(bir_json, tmpdir, neff_name="file.neff"):
        try:
            cdir = os.path.expanduser("~/.cache/bass_neff")
            os.makedirs(cdir, exist_ok=True)
            key = hashlib.sha256(
                bir_json if isinstance(bir_json, bytes) else bir_json.encode()
            ).hexdigest()[:32]
            cpath = os.path.join(cdir, f"{key}.neff")
            if os.path.exists(cpath):
                outdir = os.path.join(tmpdir, "sg00")
                os.makedirs(outdir, exist_ok=True)
                dst = os.path.join(outdir, neff_name)
                shutil.copyfile(cpath, dst)
                return dst
            neff = orig(bir_json, tmpdir, neff_name)
            shutil.copyfile(neff, cpath + ".tmp")
            os.replace(cpath + ".tmp", cpath)
            return neff
        except Exception:
            return orig(bir_json, tmpdir, neff_name)

    bass2jax.compile_bir_kernel = cached
    bass2jax._ant_neff_cache_installed = True


_install_neff_disk_cache()

F32 = mybir.dt.float32
F32R = mybir.dt.float32r

MM_DT = F32R  # matmul operand dtype: F32R (fast) or F32 (exact)

NCORES = 8
FULL_B, FULL_H, FULL_W = 16, 2048, 2048

TRACE = False  # set True (from test harness) to capture an NTFF profile
LAST_RESULTS = None  # BassKernelResults of the most recent run


def _build_bands(w1, w2, h, s, nb):
    """Host-side band matrices for the vertical taps.

    B1: [128, 3 variants, 3 dx, 126]; variant 0 = top block, 1 = mid,
    2 = bottom block.  B2: [126, 3 dx, 124].
    """
    W1 = np.asarray(w1, np.float32).reshape(3, 3)
    W2 = np.asarray(w2, np.float32).reshape(3, 3)
    m1, m2 = s + 2, s
    b1 = np.zeros((128, 3, 3, m1), np.float32)
    b2 = np.zeros((m1, 3, m2), np.float32)
    mm = np.arange(m1)
    for i in range(3):
        for dx in range(3):
            b1[mm + i, :, dx, mm] = W1[i, dx]
    mm = np.arange(m2)
    for i in range(3):
        for dx in range(3):
            b2[mm + i, dx, mm] = W2[i, dx]
    # top block: y row r0-1 = -1 is conv2's zero padding, not a computed row
    b1[:, 0, :, 0] = 0.0
    # bottom block: y row == h is zero padding
    r0_last = (nb - 1) * s
    b1[:, 2, :, h - r0_last + 1] = 0.0
    return np.ascontiguousarray(b1.reshape(128, 9 * m1)), np.ascontiguousarray(
        b2.reshape(m1, 3 * m2)
    )


def build_nc(imgs, h, w, nw=512, s=124, repeat=1):
    """Build the per-core Bass program (parametric so a small config can
    be validated in CoreSim)."""
    assert w % nw == 0 and nw <= 512
    nb = -(-h // s)  # blocks per image
    m1, m2 = s + 2, s  # conv1/conv2 output rows per block
    nch = w // nw  # width chunks
    xw = w + 4  # tile width: col 0 zero | 1..w data | w+1 zero | pad
    r0_last = (nb - 1) * s
    rows_last = h - (r0_last - 2)  # x rows loaded for the last block
    k1_last = rows_last + 2

    nc = bacc.Bacc("TRN2", target_bir_lowering=False, debug=False)
    x_d = nc.dram_tensor("x", [imgs, h, w], MM_DT, kind="ExternalInput")
    b1_d = nc.dram_tensor("b1", [128, 9 * m1], MM_DT, kind="ExternalInput")
    b2_d = nc.dram_tensor("b2", [m1, 3 * m2], MM_DT, kind="ExternalInput")
    z_d = nc.dram_tensor("z", [imgs, h, w], F32, kind="ExternalOutput")

    blocks = [(g, b) for g in range(imgs) for b in range(nb)] * repeat

    with tile.TileContext(nc) as tc:
        with (
            tc.tile_pool(name="const", bufs=1) as cpool,
            tc.tile_pool(name="xp", bufs=3) as xpool,
            tc.tile_pool(name="yp", bufs=2) as ypool,
            tc.tile_pool(name="zp", bufs=2) as zpool,
            tc.tile_pool(name="pyp", bufs=4, space="PSUM") as pypool,
            tc.tile_pool(name="pzp", bufs=4, space="PSUM") as pzpool,
        ):
            b1_t = cpool.tile([128, 9 * m1], MM_DT)
            b2_t = cpool.tile([m1, 3 * m2], MM_DT)
            nc.sync.dma_start(out=b1_t[:], in_=b1_d[:])
            nc.sync.dma_start(out=b2_t[:], in_=b2_d[:])

            pend = None  # (img, block, y_tile) awaiting conv2
            for t in range(len(blocks) + 1):
                if t < len(blocks):
                    g, b = blocks[t]
                    r0 = b * s
                    lo, hi = max(r0 - 2, 0), min(r0 + m1, h)
                    p0, rows = lo - (r0 - 2), hi - lo
                    x_t = xpool.tile([128, xw], MM_DT, tag="x")
                    if b == nb - 1:
                        # bottom block: zero the whole tile first (covers the
                        # 2 halo rows below the image and the halo columns);
                        # compute-engine APs can only start at partition
                        # 0/32/64/96, so a targeted halo-row memset is not
                        # expressible.
                        nc.vector.memzero(x_t[:, :])
                    nc.sync.dma_start(
                        out=x_t[p0 : p0 + rows, 1 : 1 + w], in_=x_d[g, lo:hi, :]
                    )
                    if b == 0:
                        nc.vector.memzero(x_t[0:2, :])
                    if b != nb - 1:
                        nc.vector.memzero(x_t[:, 0:1])
                        nc.vector.memzero(x_t[:, 1 + w : 2 + w])
                    k1 = k1_last if b == nb - 1 else 128
                    v = 0 if b == 0 else (2 if b == nb - 1 else 1)
                    y_t = ypool.tile([m1, xw], MM_DT, tag="y")
                    for j in range(nch):
                        py = pypool.tile([m1, nw], F32, tag="py")
                        for dx in range(3):
                            nc.tensor.matmul(
                                py[:],
                                b1_t[0:k1, (v * 3 + dx) * m1 : (v * 3 + dx + 1) * m1],
                                x_t[0:k1, nw * j + dx : nw * j + dx + nw],
                                start=(dx == 0),
                                stop=(dx == 2),
                            )
                        nc.vector.tensor_copy(
                            out=y_t[:, 1 + nw * j : 1 + nw * j + nw], in_=py[:]
                        )
                    nc.vector.memzero(y_t[:, 0:1])
                    nc.vector.memzero(y_t[:, 1 + w : 2 + w])
                    pend_next = (g, b, y_t)
                else:
                    pend_next = None

                if pend is not None:
                    g2, b2i, y_prev = pend
                    r0 = b2i * s
                    rows = min(s, h - r0)
                    z_t = zpool.tile([m2, w], F32, tag="z")
                    for j in range(nch):
                        pz = pzpool.tile([m2, nw], F32, tag="pz")
                        for dx in range(3):
                            nc.tensor.matmul(
                                pz[:],
                                b2_t[0:m1, dx * m2 : (dx + 1) * m2],
                                y_prev[0:m1, nw * j + dx : nw * j + dx + nw],
                                start=(dx == 0),
                                stop=(dx == 2),
                            )
                        nc.scalar.copy(out=z_t[:, nw * j : nw * j + nw], in_=pz[:])
                    nc.sync.dma_start(out=z_d[g2, r0 : r0 + rows, :], in_=z_t[0:rows, :])
                pend = pend_next

    nc.compile()
    return nc


def _build_bands5(w1, w2, h, s, nb):
    """Composite single-pass operator: z = C(x) where C = conv2 o conv1
    with the chained-SAME-padding semantics folded in exactly.

    Vertical behavior (including the y[-1]/y[h] zero rows and the image
    top/bottom) is encoded in per-variant 5-diagonal band matrices
    C[k, v, dx, m].  The only horizontal discrepancy of the composite
    vs the chained convs is the phantom y column at each side; D holds
    the two exact correction bands (applied to x col 0 / w-1, adding
    into z col 0 / w-1).
    """
    W1 = np.asarray(w1, np.float64).reshape(3, 3)
    W2 = np.asarray(w2, np.float64).reshape(3, 3)
    m1, m2 = s + 2, s

    def a_mat(col, rows, cols):
        a = np.zeros((rows, cols), np.float64)
        r = np.arange(rows)
        for i in range(3):
            a[r, r + i] = col[i]
        return a

    r0_last = (nb - 1) * s
    c = np.zeros((128, 3, 5, m2), np.float64)
    d = np.zeros((128, 3, 2, m2), np.float64)
    for v in range(3):
        a1 = [a_mat(W1[:, j], m1, 128) for j in range(3)]
        if v == 0:
            for a in a1:
                a[0, :] = 0.0  # y row -1 is conv2 zero padding
        if v == 2:
            for a in a1:
                a[h - r0_last + 1, :] = 0.0  # y row h is zero padding
        a2 = [a_mat(W2[:, j], m2, m1) for j in range(3)]
        for j in range(3):
            for jp in range(3):
                c[:, v, j + jp, :] += (a2[jp] @ a1[j]).T
        d[:, v, 0, :] = -(a2[0] @ a1[2]).T
        d[:, v, 1, :] = -(a2[2] @ a1[0]).T
    return (
        np.ascontiguousarray(c.reshape(128, 15 * m2).astype(np.float32)),
        np.ascontiguousarray(d.reshape(128, 6 * m2).astype(np.float32)),
    )


def build_nc5(imgs, h, w, nw=512, s=124, repeat=1, xbufs=4, zbufs=3, pzbufs=8, zdma="sync", corr=True):
    """Single-pass composite-5x5 program (see _build_bands5)."""
    assert w % nw == 0 and nw <= 512
    nb = -(-h // s)
    m2 = s
    nch = w // nw
    xw = w + 4  # cols 0,1 zero | 2..w+1 data | w+2,w+3 zero
    r0_last = (nb - 1) * s
    rows_last = h - (r0_last - 2)
    k_last = rows_last + 2

    nc = bacc.Bacc("TRN2", target_bir_lowering=False, debug=False)
    x_d = nc.dram_tensor("x", [imgs, h, w], MM_DT, kind="ExternalInput")
    c_d = nc.dram_tensor("b1", [128, 15 * m2], MM_DT, kind="ExternalInput")
    d_d = nc.dram_tensor("b2", [128, 6 * m2], MM_DT, kind="ExternalInput")
    z_d = nc.dram_tensor("z", [imgs, h, w], F32, kind="ExternalOutput")

    blocks = [(g, b) for g in range(imgs) for b in range(nb)] * repeat

    with tile.TileContext(nc) as tc:
        with (
            tc.tile_pool(name="const", bufs=1) as cpool,
            tc.tile_pool(name="xp", bufs=xbufs) as xpool,
            tc.tile_pool(name="zp", bufs=zbufs) as zpool,
            tc.tile_pool(name="pzp", bufs=pzbufs, space="PSUM") as pzpool,
        ):
            c_t = cpool.tile([128, 15 * m2], MM_DT)
            d_t = cpool.tile([128, 6 * m2], MM_DT)
            nc.sync.dma_start(out=c_t[:], in_=c_d[:])
            nc.sync.dma_start(out=d_t[:], in_=d_d[:])

            for g, b in blocks:
                r0 = b * s
                lo, hi = max(r0 - 2, 0), min(r0 + s + 2, h)
                p0, rows = lo - (r0 - 2), hi - lo
                x_t = xpool.tile([128, xw], MM_DT, tag="x")
                if b == nb - 1:
                    nc.vector.memzero(x_t[:, :])
                nc.sync.dma_start(
                    out=x_t[p0 : p0 + rows, 2 : 2 + w], in_=x_d[g, lo:hi, :]
                )
                if b == 0:
                    nc.vector.memzero(x_t[0:2, :])
                if b != nb - 1:
                    nc.vector.memzero(x_t[:, 0:2])
                    nc.vector.memzero(x_t[:, 2 + w : 4 + w])
                k = k_last if b == nb - 1 else 128
                v = 0 if b == 0 else (2 if b == nb - 1 else 1)
                rows_out = min(s, h - r0)
                z_t = zpool.tile([m2, w], F32, tag="z")
                for j in range(nch):
                    pz = pzpool.tile([m2, nw], F32, tag="pz")
                    corrj = corr and ((j == 0) or (j == nch - 1))
                    for dx in range(5):
                        nc.tensor.matmul(
                            pz[:],
                            c_t[0:k, (v * 5 + dx) * m2 : (v * 5 + dx + 1) * m2],
                            x_t[0:k, nw * j + dx : nw * j + dx + nw],
                            start=(dx == 0),
                            stop=(dx == 4 and not corrj),
                        )
                    # fp32r matmuls need an even moving-operand count and an
                    # 8B-aligned even-count dst, so the 1-column corrections
                    # run as N=2 with the partner column reading a zeroed
                    # halo column of x (negative-step AP) -> contributes 0.
                    if corrj and j == 0:
                        nc.tensor.matmul(
                            pz[:, 0:2],
                            d_t[0:k, (v * 2 + 0) * m2 : (v * 2 + 1) * m2],
                            x_t[0:k, 2:0:-1],  # cols [x 0, zero]
                            start=False,
                            stop=(j != nch - 1),
                        )
                    if corrj and j == nch - 1:
                        nc.tensor.matmul(
                            pz[:, nw - 2 : nw],
                            d_t[0:k, (v * 2 + 1) * m2 : (v * 2 + 2) * m2],
                            x_t[0:k, w + 2 : w : -1],  # cols [zero, x w-1]
                            start=False,
                            stop=True,
                        )
                    if j % 2 == 0:
                        nc.scalar.copy(out=z_t[:, nw * j : nw * j + nw], in_=pz[:])
                    else:
                        nc.vector.tensor_copy(
                            out=z_t[:, nw * j : nw * j + nw], in_=pz[:]
                        )
                zeng = nc.scalar if zdma == "scalar" else nc.sync
                zeng.dma_start(
                    out=z_d[g, r0 : r0 + rows_out, :], in_=z_t[0:rows_out, :]
                )

    nc.compile()
    return nc


def build_nc6(
    imgs, h, w, nw=512, s=124, repeat=1, xbufs=4, zbufs=3, pzbufs=8, zdma="scalar"
):
    """Composite single-pass program over HOST-PADDED x.

    x arrives as [imgs, h+4, w+4] with 2 zero rows/cols on every side,
    so the device needs no halo memsets at all: each band of s output
    rows is one clean [<=128, w+4] DMA, 22 accumulating fp32r matmuls,
    4 PSUM->SBUF copies and one store."""
    assert w % nw == 0 and nw <= 512
    nb = -(-h // s)
    m2 = s
    nch = w // nw
    xw = w + 4
    r0_last = (nb - 1) * s
    k_last = h + 4 - r0_last  # padded rows available for the last block

    nc = bacc.Bacc("TRN2", target_bir_lowering=False, debug=False)
    x_d = nc.dram_tensor("x", [imgs, h + 4, w + 4], MM_DT, kind="ExternalInput")
    c_d = nc.dram_tensor("b1", [128, 15 * m2], MM_DT, kind="ExternalInput")
    d_d = nc.dram_tensor("b2", [128, 6 * m2], MM_DT, kind="ExternalInput")
    z_d = nc.dram_tensor("z", [imgs, h, w], F32, kind="ExternalOutput")

    blocks = [(g, b) for g in range(imgs) for b in range(nb)] * repeat

    with tile.TileContext(nc) as tc:
        with (
            tc.tile_pool(name="const", bufs=1) as cpool,
            tc.tile_pool(name="xp", bufs=xbufs) as xpool,
            tc.tile_pool(name="zp", bufs=zbufs) as zpool,
            tc.tile_pool(name="pzp", bufs=pzbufs, space="PSUM") as pzpool,
        ):
            c_t = cpool.tile([128, 15 * m2], MM_DT)
            d_t = cpool.tile([128, 6 * m2], MM_DT)
            nc.sync.dma_start(out=c_t[:], in_=c_d[:])
            nc.sync.dma_start(out=d_t[:], in_=d_d[:])

            for g, b in blocks:
                r0 = b * s
                k = k_last if b == nb - 1 else 128
                x_t = xpool.tile([128, xw], MM_DT, tag="x")
                nc.sync.dma_start(out=x_t[0:k, :], in_=x_d[g, r0 : r0 + k, :])
                v = 0 if b == 0 else (2 if b == nb - 1 else 1)
                rows_out = min(s, h - r0)
                z_t = zpool.tile([m2, w], F32, tag="z")
                for j in range(nch):
                    pz = pzpool.tile([m2, nw], F32, tag="pz")
                    corrj = (j == 0) or (j == nch - 1)
                    for dx in range(5):
                        nc.tensor.matmul(
                            pz[:],
                            c_t[0:k, (v * 5 + dx) * m2 : (v * 5 + dx + 1) * m2],
                            x_t[0:k, nw * j + dx : nw * j + dx + nw],
                            start=(dx == 0),
                            stop=(dx == 4 and not corrj),
                        )
                    if j == 0:
                        nc.tensor.matmul(
                            pz[:, 0:2],
                            d_t[0:k, (v * 2 + 0) * m2 : (v * 2 + 1) * m2],
                            x_t[0:k, 2:0:-1],  # cols [x 0, zero]
                            start=False,
                            stop=(j != nch - 1),
                        )
                    if j == nch - 1:
                        nc.tensor.matmul(
                            pz[:, nw - 2 : nw],
                            d_t[0:k, (v * 2 + 1) * m2 : (v * 2 + 2) * m2],
                            x_t[0:k, w + 2 : w : -1],  # cols [zero, x w-1]
                            start=False,
                            stop=True,
                        )
                    if j % 2 == 0:
                        nc.scalar.copy(out=z_t[:, nw * j : nw * j + nw], in_=pz[:])
                    else:
                        nc.vector.tensor_copy(
                            out=z_t[:, nw * j : nw * j + nw], in_=pz[:]
                        )
                zeng = nc.scalar if zdma == "scalar" else nc.sync
                zeng.dma_start(
                    out=z_d[g, r0 : r0 + rows_out, :], in_=z_t[0:rows_out, :]
                )

    nc.compile()
    return nc


def pad_x(x, imgs, h, w):
    xp = np.zeros((imgs, h + 4, w + 4), np.float32)
    xp[:, 2 : h + 2, 2 : w + 2] = x
    return xp


BF16 = mybir.dt.bfloat16


def build_nc7(imgs, h, w, nw=512, s=124, hwloop=1, xbufs=6, zbufs=4, pzbufs=8):
    """bf16 single-pass composite-5x5 program (see _build_bands5).

    vs build_nc5: x/z/bands in bf16 (halves HBM traffic; PSUM accumulation
    stays f32), z stores issued from the Activation engine so loads (SP
    ring) and stores (ACT ring) use the two independent HW DGE queues, and
    an optional hardware loop (`hwloop` passes per NEFF execution) for
    dispatch-overhead-free steady-state timing.
    """
    assert w % nw == 0 and nw <= 512
    nb = -(-h // s)
    m2 = s
    nch = w // nw
    xw = w + 4  # cols 0,1 zero | 2..w+1 data | w+2,w+3 zero
    r0_last = (nb - 1) * s
    rows_last = h - (r0_last - 2)
    k_last = rows_last + 2

    nc = bacc.Bacc("TRN2", target_bir_lowering=False, debug=False)
    x_d = nc.dram_tensor("x", [imgs, h, w], BF16, kind="ExternalInput")
    c_d = nc.dram_tensor("b1", [128, 15 * m2], BF16, kind="ExternalInput")
    d_d = nc.dram_tensor("b2", [128, 6 * m2], BF16, kind="ExternalInput")
    z_d = nc.dram_tensor("z", [imgs, h, w], BF16, kind="ExternalOutput")

    blocks = [(g, b) for g in range(imgs) for b in range(nb)]

    with tile.TileContext(nc) as tc:
        with (
            tc.tile_pool(name="const", bufs=1) as cpool,
            tc.tile_pool(name="xp", bufs=xbufs) as xpool,
            tc.tile_pool(name="zp", bufs=zbufs) as zpool,
            tc.tile_pool(name="pzp", bufs=pzbufs, space="PSUM") as pzpool,
        ):
            c_t = cpool.tile([128, 15 * m2], BF16)
            d_t = cpool.tile([128, 6 * m2], BF16)
            nc.sync.dma_start(out=c_t[:], in_=c_d[:])
            nc.sync.dma_start(out=d_t[:], in_=d_d[:])

            def body():
                for g, b in blocks:
                    r0 = b * s
                    lo, hi = max(r0 - 2, 0), min(r0 + s + 2, h)
                    p0, rows = lo - (r0 - 2), hi - lo
                    x_t = xpool.tile([128, xw], BF16, tag="x")
                    if b == nb - 1:
                        nc.vector.memzero(x_t[:, :])
                    nc.sync.dma_start(
                        out=x_t[p0 : p0 + rows, 2 : 2 + w], in_=x_d[g, lo:hi, :]
                    )
                    if b == 0:
                        nc.vector.memzero(x_t[0:2, :])
                    if b != nb - 1:
                        nc.vector.memzero(x_t[:, 0:2])
                        nc.vector.memzero(x_t[:, 2 + w : 4 + w])
                    k = k_last if b == nb - 1 else 128
                    v = 0 if b == 0 else (2 if b == nb - 1 else 1)
                    rows_out = min(s, h - r0)
                    z_t = zpool.tile([m2, w], BF16, tag="z")
                    for j in range(nch):
                        pz = pzpool.tile([m2, nw], F32, tag="pz")
                        corrj = (j == 0) or (j == nch - 1)
                        for dx in range(5):
                            nc.tensor.matmul(
                                pz[:],
                                c_t[0:k, (v * 5 + dx) * m2 : (v * 5 + dx + 1) * m2],
                                x_t[0:k, nw * j + dx : nw * j + dx + nw],
                                start=(dx == 0),
                                stop=(dx == 4 and not corrj),
                            )
                        if corrj and j == 0:
                            nc.tensor.matmul(
                                pz[:, 0:2],
                                d_t[0:k, (v * 2 + 0) * m2 : (v * 2 + 1) * m2],
                                x_t[0:k, 2:0:-1],  # cols [x 0, zero]
                                start=False,
                                stop=(j != nch - 1),
                            )
                        if corrj and j == nch - 1:
                            nc.tensor.matmul(
                                pz[:, nw - 2 : nw],
                                d_t[0:k, (v * 2 + 1) * m2 : (v * 2 + 2) * m2],
                                x_t[0:k, w + 2 : w : -1],  # cols [zero, x w-1]
                                start=False,
                                stop=True,
                            )
                        if j % 2 == 0:
                            nc.scalar.copy(out=z_t[:, nw * j : nw * j + nw], in_=pz[:])
                        else:
                            nc.vector.tensor_copy(
                                out=z_t[:, nw * j : nw * j + nw], in_=pz[:]
                            )
                    nc.scalar.dma_start(
                        out=z_d[g, r0 : r0 + rows_out, :], in_=z_t[0:rows_out, :]
                    )

            if hwloop > 1:
                with tc.For_i(0, hwloop):
                    body()
            else:
                body()

    nc.compile()
    return nc


def build_nc8(imgs, h, w, nw=512, s=124, hwloop=1, xbufs=6, zbufs=4, pzbufs=8):
    """build_nc7 with the stationary band sections zero-padded from m2=124
    to 128 columns. A 128-column weight load triggers the PE's automatic
    Fast Weight Load path (2x for bf16); PSUM tiles grow to 128 partitions
    (rows 124..127 compute zeros) but still fit one 2KB bank."""
    assert w % nw == 0 and nw <= 512
    nb = -(-h // s)
    m2 = s
    ms = 128  # padded stationary columns / PSUM partitions
    nch = w // nw
    xw = w + 4
    r0_last = (nb - 1) * s
    rows_last = h - (r0_last - 2)
    k_last = rows_last + 2

    nc = bacc.Bacc("TRN2", target_bir_lowering=False, debug=False)
    x_d = nc.dram_tensor("x", [imgs, h, w], BF16, kind="ExternalInput")
    c_d = nc.dram_tensor("b1", [128, 15 * ms], BF16, kind="ExternalInput")
    d_d = nc.dram_tensor("b2", [128, 6 * ms], BF16, kind="ExternalInput")
    z_d = nc.dram_tensor("z", [imgs, h, w], BF16, kind="ExternalOutput")

    blocks = [(g, b) for g in range(imgs) for b in range(nb)]

    with tile.TileContext(nc) as tc:
        with (
            tc.tile_pool(name="const", bufs=1) as cpool,
            tc.tile_pool(name="xp", bufs=xbufs) as xpool,
            tc.tile_pool(name="zp", bufs=zbufs) as zpool,
            tc.tile_pool(name="pzp", bufs=pzbufs, space="PSUM") as pzpool,
        ):
            c_t = cpool.tile([128, 15 * ms], BF16)
            d_t = cpool.tile([128, 6 * ms], BF16)
            nc.sync.dma_start(out=c_t[:], in_=c_d[:])
            nc.sync.dma_start(out=d_t[:], in_=d_d[:])

            def body():
                for g, b in blocks:
                    r0 = b * s
                    lo, hi = max(r0 - 2, 0), min(r0 + s + 2, h)
                    p0, rows = lo - (r0 - 2), hi - lo
                    x_t = xpool.tile([128, xw], BF16, tag="x")
                    if b == nb - 1:
                        # only partitions >= rows hold stale data; compute-engine
                        # APs start at partition 0/32/64/96 and must not span
                        # more partitions than their alignment allows
                        zp0 = max(p for p in (0, 32, 64, 96) if p <= rows)
                        for q0, q1 in ((0, 32), (32, 64), (64, 128)):
                            if q1 > zp0:
                                nc.vector.memzero(x_t[max(q0, zp0) : q1, :])
                    nc.sync.dma_start(
                        out=x_t[p0 : p0 + rows, 2 : 2 + w], in_=x_d[g, lo:hi, :]
                    )
                    if b == 0:
                        nc.vector.memzero(x_t[0:2, :])
                    nc.vector.memzero(x_t[:, 0:2])
                    nc.vector.memzero(x_t[:, 2 + w : 4 + w])
                    k = k_last if b == nb - 1 else 128
                    v = 0 if b == 0 else (2 if b == nb - 1 else 1)
                    rows_out = min(s, h - r0)
                    z_t = zpool.tile([m2, w], BF16, tag="z")
                    for j in range(nch):
                        pz = pzpool.tile([ms, nw], F32, tag="pz")
                        corrj = (j == 0) or (j == nch - 1)
                        for dx in range(5):
                            nc.tensor.matmul(
                                pz[:],
                                c_t[0:k, (v * 5 + dx) * ms : (v * 5 + dx + 1) * ms],
                                x_t[0:k, nw * j + dx : nw * j + dx + nw],
                                start=(dx == 0),
                                stop=(dx == 4 and not corrj),
                            )
                        if corrj and j == 0:
                            nc.tensor.matmul(
                                pz[:, 0:2],
                                d_t[0:k, (v * 2 + 0) * ms : (v * 2 + 1) * ms],
                                x_t[0:k, 2:0:-1],  # cols [x 0, zero]
                                start=False,
                                stop=(j != nch - 1),
                            )
                        if corrj and j == nch - 1:
                            nc.tensor.matmul(
                                pz[:, nw - 2 : nw],
                                d_t[0:k, (v * 2 + 1) * ms : (v * 2 + 2) * ms],
                                x_t[0:k, w + 2 : w : -1],  # cols [zero, x w-1]
                                start=False,
                                stop=True,
                            )
                        if j % 2 == 0:
                            nc.scalar.copy(
                                out=z_t[:, nw * j : nw * j + nw], in_=pz[0:m2, :]
                            )
                        else:
                            nc.vector.tensor_copy(
                                out=z_t[:, nw * j : nw * j + nw], in_=pz[0:m2, :]
                            )
                    nc.scalar.dma_start(
                        out=z_d[g, r0 : r0 + rows_out, :], in_=z_t[0:rows_out, :]
                    )

            if hwloop > 1:
                with tc.For_i(0, hwloop):
                    body()
            else:
                body()

    nc.compile()
    return nc


def _pad_bands_128(b1, b2, m2):
    c = b1.reshape(128, 15, m2)
    cp = np.zeros((128, 15, 128), b1.dtype)
    cp[:, :, :m2] = c
    d = b2.reshape(128, 6, m2)
    dp = np.zeros((128, 6, 128), b2.dtype)
    dp[:, :, :m2] = d
    return (
        np.ascontiguousarray(cp.reshape(128, 15 * 128)),
        np.ascontiguousarray(dp.reshape(128, 6 * 128)),
    )


def to_bf16(a):
    import ml_dtypes

    return np.ascontiguousarray(np.asarray(a).astype(ml_dtypes.bfloat16))


def make_in_maps(x, w1, w2, h=FULL_H, s=124):
    """bf16 per-core input maps for build_nc8 from full f32 inputs."""
    nb = -(-h // s)
    b1, b2 = _build_bands5(w1, w2, h, s, nb)
    b1, b2 = _pad_bands_128(b1, b2, s)
    b1, b2 = to_bf16(b1), to_bf16(b2)
    xb = to_bf16(np.asarray(x, np.float32).reshape(FULL_B, FULL_H, FULL_W))
    imgs = FULL_B // NCORES
    return [
        {"x": np.ascontiguousarray(xb[imgs * c : imgs * (c + 1)]), "b1": b1, "b2": b2}
        for c in range(NCORES)
    ]


_NC_CACHE = None


def kernel(x, w1, w2):
    global _NC_CACHE, LAST_RESULTS
    in_maps = make_in_maps(x, w1, w2)
    if _NC_CACHE is None:
        _NC_CACHE = build_nc8(FULL_B // NCORES, FULL_H, FULL_W, nw=512, s=124)
    nc = _NC_CACHE
    res = run_bass_kernel_spmd(nc, in_maps, core_ids=list(range(NCORES)), trace=TRACE)
    LAST_RESULTS = res
    out = np.stack(
        [np.asarray(res.results[c]["z"], np.float32) for c in range(NCORES)], axis=0
    )
    return out.reshape(FULL_B, 1, FULL_H, FULL_W)



# revision 14
# speedup vs baseline: 1.1778x; 1.1778x over previous
"""Trainium2 Bass kernel: two chained SAME-padded 3x3 single-channel convs.

  reference: z = conv3x3(conv3x3(x, w1), w2)   x: [16,1,2048,2048] f32

Strategy (pure data parallel, 2 images per core on 8 cores):
  - Images processed in horizontal bands of S=124 output rows.
  - Each conv is computed on the TensorEngine as 3 banded matmuls
    (one per kernel column dx), accumulating in PSUM. The band matrix
    B_dx[k, m] = W[k-m, dx] applies the vertical taps; the horizontal
    taps come from shifting the moving operand (rhs) by dx columns.
  - conv1 output (y) is copied PSUM->SBUF (VectorE) and consumed by
    conv2's matmuls; conv2 output is copied PSUM->SBUF (ScalarE) and
    DMA'd out. Intermediate y never touches HBM.
  - float32r matmuls (1 cycle/row on the PE vs 4 for fp32; ~1.5e-4
    absmax-relative rounding measured on HW). Set MM_DT = F32 below for
    full fp32 precision at ~4x the PE cost.
  - SAME padding handled with zeroed halo columns in SBUF and
    host-built band-matrix variants for the top/bottom image edges.

Band matrices are built on the host from w1/w2 (they are just 9 floats
each) and passed as extra inputs.
"""

import hashlib
import os
import shutil

import numpy as np

import concourse.mybir as mybir
import concourse.tile as tile
from concourse import bacc, bass2jax
from concourse.bass_utils import run_bass_kernel_spmd


def _install_neff_disk_cache():
    """Cache compiled NEFFs on disk keyed by BIR content hash — the
    neuronxcc backend takes minutes for this kernel and has no cache of
    its own, so a fresh process would otherwise recompile every run."""
    if getattr(bass2jax, "_ant_neff_cache_installed", False):
        return
    orig = bass2jax.compile_bir_kernel

    def cached(bir_json, tmpdir, neff_name="file.neff"):
        try:
            cdir = os.path.expanduser("~/.cache/bass_neff")
            os.makedirs(cdir, exist_ok=True)
            key = hashlib.sha256(
                bir_json if isinstance(bir_json, bytes) else bir_json.encode()
            ).hexdigest()[:32]
            cpath = os.path.join(cdir, f"{key}.neff")
            if os.path.exists(cpath):
                outdir = os.path.join(tmpdir, "sg00")
                os.makedirs(outdir, exist_ok=True)
                dst = os.path.join(outdir, neff_name)
                shutil.copyfile(cpath, dst)
                return dst
            neff = orig(bir_json, tmpdir, neff_name)
            shutil.copyfile(neff, cpath + ".tmp")
            os.replace(cpath + ".tmp", cpath)
            return neff
        except Exception:
            return orig(bir_json, tmpdir, neff_name)

    bass2jax.compile_bir_kernel = cached
    bass2jax._ant_neff_cache_installed = True


_install_neff_disk_cache()

F32 = mybir.dt.float32
F32R = mybir.dt.float32r

MM_DT = F32R  # matmul operand dtype: F32R (fast) or F32 (exact)

NCORES = 8
FULL_B, FULL_H, FULL_W = 16, 2048, 2048

TRACE = False  # set True (from test harness) to capture an NTFF profile
LAST_RESULTS = None  # BassKernelResults of the most recent run


def _build_bands(w1, w2, h, s, nb):
    """Host-side band matrices for the vertical taps.

    B1: [128, 3 variants, 3 dx, 126]; variant 0 = top block, 1 = mid,
    2 = bottom block.  B2: [126, 3 dx, 124].
    """
    W1 = np.asarray(w1, np.float32).reshape(3, 3)
    W2 = np.asarray(w2, np.float32).reshape(3, 3)
    m1, m2 = s + 2, s
    b1 = np.zeros((128, 3, 3, m1), np.float32)
    b2 = np.zeros((m1, 3, m2), np.float32)
    mm = np.arange(m1)
    for i in range(3):
        for dx in range(3):
            b1[mm + i, :, dx, mm] = W1[i, dx]
    mm = np.arange(m2)
    for i in range(3):
        for dx in range(3):
            b2[mm + i, dx, mm] = W2[i, dx]
    # top block: y row r0-1 = -1 is conv2's zero padding, not a computed row
    b1[:, 0, :, 0] = 0.0
    # bottom block: y row == h is zero padding
    r0_last = (nb - 1) * s
    b1[:, 2, :, h - r0_last + 1] = 0.0
    return np.ascontiguousarray(b1.reshape(128, 9 * m1)), np.ascontiguousarray(
        b2.reshape(m1, 3 * m2)
    )


def build_nc(imgs, h, w, nw=512, s=124, repeat=1):
    """Build the per-core Bass program (parametric so a small config can
    be validated in CoreSim)."""
    assert w % nw == 0 and nw <= 512
    nb = -(-h // s)  # blocks per image
    m1, m2 = s + 2, s  # conv1/conv2 output rows per block
    nch = w // nw  # width chunks
    xw = w + 4  # tile width: col 0 zero | 1..w data | w+1 zero | pad
    r0_last = (nb - 1) * s
    rows_last = h - (r0_last - 2)  # x rows loaded for the last block
    k1_last = rows_last + 2

    nc = bacc.Bacc("TRN2", target_bir_lowering=False, debug=False)
    x_d = nc.dram_tensor("x", [imgs, h, w], MM_DT, kind="ExternalInput")
    b1_d = nc.dram_tensor("b1", [128, 9 * m1], MM_DT, kind="ExternalInput")
    b2_d = nc.dram_tensor("b2", [m1, 3 * m2], MM_DT, kind="ExternalInput")
    z_d = nc.dram_tensor("z", [imgs, h, w], F32, kind="ExternalOutput")

    blocks = [(g, b) for g in range(imgs) for b in range(nb)] * repeat

    with tile.TileContext(nc) as tc:
        with (
            tc.tile_pool(name="const", bufs=1) as cpool,
            tc.tile_pool(name="xp", bufs=3) as xpool,
            tc.tile_pool(name="yp", bufs=2) as ypool,
            tc.tile_pool(name="zp", bufs=2) as zpool,
            tc.tile_pool(name="pyp", bufs=4, space="PSUM") as pypool,
            tc.tile_pool(name="pzp", bufs=4, space="PSUM") as pzpool,
        ):
            b1_t = cpool.tile([128, 9 * m1], MM_DT)
            b2_t = cpool.tile([m1, 3 * m2], MM_DT)
            nc.sync.dma_start(out=b1_t[:], in_=b1_d[:])
            nc.sync.dma_start(out=b2_t[:], in_=b2_d[:])

            pend = None  # (img, block, y_tile) awaiting conv2
            for t in range(len(blocks) + 1):
                if t < len(blocks):
                    g, b = blocks[t]
                    r0 = b * s
                    lo, hi = max(r0 - 2, 0), min(r0 + m1, h)
                    p0, rows = lo - (r0 - 2), hi - lo
                    x_t = xpool.tile([128, xw], MM_DT, tag="x")
                    if b == nb - 1:
                        # bottom block: zero the whole tile first (covers the
                        # 2 halo rows below the image and the halo columns);
                        # compute-engine APs can only start at partition
                        # 0/32/64/96, so a targeted halo-row memset is not
                        # expressible.
                        nc.vector.memzero(x_t[:, :])
                    nc.sync.dma_start(
                        out=x_t[p0 : p0 + rows, 1 : 1 + w], in_=x_d[g, lo:hi, :]
                    )
                    if b == 0:
                        nc.vector.memzero(x_t[0:2, :])
                    if b != nb - 1:
                        nc.vector.memzero(x_t[:, 0:1])
                        nc.vector.memzero(x_t[:, 1 + w : 2 + w])
                    k1 = k1_last if b == nb - 1 else 128
                    v = 0 if b == 0 else (2 if b == nb - 1 else 1)
                    y_t = ypool.tile([m1, xw], MM_DT, tag="y")
                    for j in range(nch):
                        py = pypool.tile([m1, nw], F32, tag="py")
                        for dx in range(3):
                            nc.tensor.matmul(
                                py[:],
                                b1_t[0:k1, (v * 3 + dx) * m1 : (v * 3 + dx + 1) * m1],
                                x_t[0:k1, nw * j + dx : nw * j + dx + nw],
                                start=(dx == 0),
                                stop=(dx == 2),
                            )
                        nc.vector.tensor_copy(
                            out=y_t[:, 1 + nw * j : 1 + nw * j + nw], in_=py[:]
                        )
                    nc.vector.memzero(y_t[:, 0:1])
                    nc.vector.memzero(y_t[:, 1 + w : 2 + w])
                    pend_next = (g, b, y_t)
                else:
                    pend_next = None

                if pend is not None:
                    g2, b2i, y_prev = pend
                    r0 = b2i * s
                    rows = min(s, h - r0)
                    z_t = zpool.tile([m2, w], F32, tag="z")
                    for j in range(nch):
                        pz = pzpool.tile([m2, nw], F32, tag="pz")
                        for dx in range(3):
                            nc.tensor.matmul(
                                pz[:],
                                b2_t[0:m1, dx * m2 : (dx + 1) * m2],
                                y_prev[0:m1, nw * j + dx : nw * j + dx + nw],
                                start=(dx == 0),
                                stop=(dx == 2),
                            )
                        nc.scalar.copy(out=z_t[:, nw * j : nw * j + nw], in_=pz[:])
                    nc.sync.dma_start(out=z_d[g2, r0 : r0 + rows, :], in_=z_t[0:rows, :])
                pend = pend_next

    nc.compile()
    return nc


def _build_bands5(w1, w2, h, s, nb):
    """Composite single-pass operator: z = C(x) where C = conv2 o conv1
    with the chained-SAME-padding semantics folded in exactly.

    Vertical behavior (including the y[-1]/y[h] zero rows and the image
    top/bottom) is encoded in per-variant 5-diagonal band matrices
    C[k, v, dx, m].  The only horizontal discrepancy of the composite
    vs the chained convs is the phantom y column at each side; D holds
    the two exact correction bands (applied to x col 0 / w-1, adding
    into z col 0 / w-1).
    """
    W1 = np.asarray(w1, np.float64).reshape(3, 3)
    W2 = np.asarray(w2, np.float64).reshape(3, 3)
    m1, m2 = s + 2, s

    def a_mat(col, rows, cols):
        a = np.zeros((rows, cols), np.float64)
        r = np.arange(rows)
        for i in range(3):
            a[r, r + i] = col[i]
        return a

    r0_last = (nb - 1) * s
    c = np.zeros((128, 3, 5, m2), np.float64)
    d = np.zeros((128, 3, 2, m2), np.float64)
    for v in range(3):
        a1 = [a_mat(W1[:, j], m1, 128) for j in range(3)]
        if v == 0:
            for a in a1:
                a[0, :] = 0.0  # y row -1 is conv2 zero padding
        if v == 2:
            for a in a1:
                a[h - r0_last + 1, :] = 0.0  # y row h is zero padding
        a2 = [a_mat(W2[:, j], m2, m1) for j in range(3)]
        for j in range(3):
            for jp in range(3):
                c[:, v, j + jp, :] += (a2[jp] @ a1[j]).T
        d[:, v, 0, :] = -(a2[0] @ a1[2]).T
        d[:, v, 1, :] = -(a2[2] @ a1[0]).T
    return (
        np.ascontiguousarray(c.reshape(128, 15 * m2).astype(np.float32)),
        np.ascontiguousarray(d.reshape(128, 6 * m2).astype(np.float32)),
    )


def build_nc5(imgs, h, w, nw=512, s=124, repeat=1, xbufs=4, zbufs=3, pzbufs=8, zdma="sync", corr=True):
    """Single-pass composite-5x5 program (see _build_bands5)."""
    assert w % nw == 0 and nw <= 512
    nb = -(-h // s)
    m2 = s
    nch = w // nw
    xw = w + 4  # cols 0,1 zero | 2..w+1 data | w+2,w+3 zero
    r0_last = (nb - 1) * s
    rows_last = h - (r0_last - 2)
    k_last = rows_last + 2

    nc = bacc.Bacc("TRN2", target_bir_lowering=False, debug=False)
    x_d = nc.dram_tensor("x", [imgs, h, w], MM_DT, kind="ExternalInput")
    c_d = nc.dram_tensor("b1", [128, 15 * m2], MM_DT, kind="ExternalInput")
    d_d = nc.dram_tensor("b2", [128, 6 * m2], MM_DT, kind="ExternalInput")
    z_d = nc.dram_tensor("z", [imgs, h, w], F32, kind="ExternalOutput")

    blocks = [(g, b) for g in range(imgs) for b in range(nb)] * repeat

    with tile.TileContext(nc) as tc:
        with (
            tc.tile_pool(name="const", bufs=1) as cpool,
            tc.tile_pool(name="xp", bufs=xbufs) as xpool,
            tc.tile_pool(name="zp", bufs=zbufs) as zpool,
            tc.tile_pool(name="pzp", bufs=pzbufs, space="PSUM") as pzpool,
        ):
            c_t = cpool.tile([128, 15 * m2], MM_DT)
            d_t = cpool.tile([128, 6 * m2], MM_DT)
            nc.sync.dma_start(out=c_t[:], in_=c_d[:])
            nc.sync.dma_start(out=d_t[:], in_=d_d[:])

            for g, b in blocks:
                r0 = b * s
                lo, hi = max(r0 - 2, 0), min(r0 + s + 2, h)
                p0, rows = lo - (r0 - 2), hi - lo
                x_t = xpool.tile([128, xw], MM_DT, tag="x")
                if b == nb - 1:
                    nc.vector.memzero(x_t[:, :])
                nc.sync.dma_start(
                    out=x_t[p0 : p0 + rows, 2 : 2 + w], in_=x_d[g, lo:hi, :]
                )
                if b == 0:
                    nc.vector.memzero(x_t[0:2, :])
                if b != nb - 1:
                    nc.vector.memzero(x_t[:, 0:2])
                    nc.vector.memzero(x_t[:, 2 + w : 4 + w])
                k = k_last if b == nb - 1 else 128
                v = 0 if b == 0 else (2 if b == nb - 1 else 1)
                rows_out = min(s, h - r0)
                z_t = zpool.tile([m2, w], F32, tag="z")
                for j in range(nch):
                    pz = pzpool.tile([m2, nw], F32, tag="pz")
                    corrj = corr and ((j == 0) or (j == nch - 1))
                    for dx in range(5):
                        nc.tensor.matmul(
                            pz[:],
                            c_t[0:k, (v * 5 + dx) * m2 : (v * 5 + dx + 1) * m2],
                            x_t[0:k, nw * j + dx : nw * j + dx + nw],
                            start=(dx == 0),
                            stop=(dx == 4 and not corrj),
                        )
                    # fp32r matmuls need an even moving-operand count and an
                    # 8B-aligned even-count dst, so the 1-column corrections
                    # run as N=2 with the partner column reading a zeroed
                    # halo column of x (negative-step AP) -> contributes 0.
                    if corrj and j == 0:
                        nc.tensor.matmul(
                            pz[:, 0:2],
                            d_t[0:k, (v * 2 + 0) * m2 : (v * 2 + 1) * m2],
                            x_t[0:k, 2:0:-1],  # cols [x 0, zero]
                            start=False,
                            stop=(j != nch - 1),
                        )
                    if corrj and j == nch - 1:
                        nc.tensor.matmul(
                            pz[:, nw - 2 : nw],
                            d_t[0:k, (v * 2 + 1) * m2 : (v * 2 + 2) * m2],
                            x_t[0:k, w + 2 : w : -1],  # cols [zero, x w-1]
                            start=False,
                            stop=True,
                        )
                    if j % 2 == 0:
                        nc.scalar.copy(out=z_t[:, nw * j : nw * j + nw], in_=pz[:])
                    else:
                        nc.vector.tensor_copy(
                            out=z_t[:, nw * j : nw * j + nw], in_=pz[:]
                        )
                zeng = nc.scalar if zdma == "scalar" else nc.sync
                zeng.dma_start(
                    out=z_d[g, r0 : r0 + rows_out, :], in_=z_t[0:rows_out, :]
                )

    nc.compile()
    return nc


def build_nc6(
    imgs, h, w, nw=512, s=124, repeat=1, xbufs=4, zbufs=3, pzbufs=8, zdma="scalar"
):
    """Composite single-pass program over HOST-PADDED x.

    x arrives as [imgs, h+4, w+4] with 2 zero rows/cols on every side,
    so the device needs no halo memsets at all: each band of s output
    rows is one clean [<=128, w+4] DMA, 22 accumulating fp32r matmuls,
    4 PSUM->SBUF copies and one store."""
    assert w % nw == 0 and nw <= 512
    nb = -(-h // s)
    m2 = s
    nch = w // nw
    xw = w + 4
    r0_last = (nb - 1) * s
    k_last = h + 4 - r0_last  # padded rows available for the last block

    nc = bacc.Bacc("TRN2", target_bir_lowering=False, debug=False)
    x_d = nc.dram_tensor("x", [imgs, h + 4, w + 4], MM_DT, kind="ExternalInput")
    c_d = nc.dram_tensor("b1", [128, 15 * m2], MM_DT, kind="ExternalInput")
    d_d = nc.dram_tensor("b2", [128, 6 * m2], MM_DT, kind="ExternalInput")
    z_d = nc.dram_tensor("z", [imgs, h, w], F32, kind="ExternalOutput")

    blocks = [(g, b) for g in range(imgs) for b in range(nb)] * repeat

    with tile.TileContext(nc) as tc:
        with (
            tc.tile_pool(name="const", bufs=1) as cpool,
            tc.tile_pool(name="xp", bufs=xbufs) as xpool,
            tc.tile_pool(name="zp", bufs=zbufs) as zpool,
            tc.tile_pool(name="pzp", bufs=pzbufs, space="PSUM") as pzpool,
        ):
            c_t = cpool.tile([128, 15 * m2], MM_DT)
            d_t = cpool.tile([128, 6 * m2], MM_DT)
            nc.sync.dma_start(out=c_t[:], in_=c_d[:])
            nc.sync.dma_start(out=d_t[:], in_=d_d[:])

            for g, b in blocks:
                r0 = b * s
                k = k_last if b == nb - 1 else 128
                x_t = xpool.tile([128, xw], MM_DT, tag="x")
                nc.sync.dma_start(out=x_t[0:k, :], in_=x_d[g, r0 : r0 + k, :])
                v = 0 if b == 0 else (2 if b == nb - 1 else 1)
                rows_out = min(s, h - r0)
                z_t = zpool.tile([m2, w], F32, tag="z")
                for j in range(nch):
                    pz = pzpool.tile([m2, nw], F32, tag="pz")
                    corrj = (j == 0) or (j == nch - 1)
                    for dx in range(5):
                        nc.tensor.matmul(
                            pz[:],
                            c_t[0:k, (v * 5 + dx) * m2 : (v * 5 + dx + 1) * m2],
                            x_t[0:k, nw * j + dx : nw * j + dx + nw],
                            start=(dx == 0),
                            stop=(dx == 4 and not corrj),
                        )
                    if j == 0:
                        nc.tensor.matmul(
                            pz[:, 0:2],
                            d_t[0:k, (v * 2 + 0) * m2 : (v * 2 + 1) * m2],
                            x_t[0:k, 2:0:-1],  # cols [x 0, zero]
                            start=False,
                            stop=(j != nch - 1),
                        )
                    if j == nch - 1:
                        nc.tensor.matmul(
                            pz[:, nw - 2 : nw],
                            d_t[0:k, (v * 2 + 1) * m2 : (v * 2 + 2) * m2],
                            x_t[0:k, w + 2 : w : -1],  # cols [zero, x w-1]
                            start=False,
                            stop=True,
                        )
                    if j % 2 == 0:
                        nc.scalar.copy(out=z_t[:, nw * j : nw * j + nw], in_=pz[:])
                    else:
                        nc.vector.tensor_copy(
                            out=z_t[:, nw * j : nw * j + nw], in_=pz[:]
                        )
                zeng = nc.scalar if zdma == "scalar" else nc.sync
                zeng.dma_start(
                    out=z_d[g, r0 : r0 + rows_out, :], in_=z_t[0:rows_out, :]
                )

    nc.compile()
    return nc


def pad_x(x, imgs, h, w):
    xp = np.zeros((imgs, h + 4, w + 4), np.float32)
    xp[:, 2 : h + 2, 2 : w + 2] = x
    return xp


BF16 = mybir.dt.bfloat16


def build_nc7(imgs, h, w, nw=512, s=124, hwloop=1, xbufs=6, zbufs=4, pzbufs=8):
    """bf16 single-pass composite-5x5 program (see _build_bands5).

    vs build_nc5: x/z/bands in bf16 (halves HBM traffic; PSUM accumulation
    stays f32), z stores issued from the Activation engine so loads (SP
    ring) and stores (ACT ring) use the two independent HW DGE queues, and
    an optional hardware loop (`hwloop` passes per NEFF execution) for
    dispatch-overhead-free steady-state timing.
    """
    assert w % nw == 0 and nw <= 512
    nb = -(-h // s)
    m2 = s
    nch = w // nw
    xw = w + 4  # cols 0,1 zero | 2..w+1 data | w+2,w+3 zero
    r0_last = (nb - 1) * s
    rows_last = h - (r0_last - 2)
    k_last = rows_last + 2

    nc = bacc.Bacc("TRN2", target_bir_lowering=False, debug=False)
    x_d = nc.dram_tensor("x", [imgs, h, w], BF16, kind="ExternalInput")
    c_d = nc.dram_tensor("b1", [128, 15 * m2], BF16, kind="ExternalInput")
    d_d = nc.dram_tensor("b2", [128, 6 * m2], BF16, kind="ExternalInput")
    z_d = nc.dram_tensor("z", [imgs, h, w], BF16, kind="ExternalOutput")

    blocks = [(g, b) for g in range(imgs) for b in range(nb)]

    with tile.TileContext(nc) as tc:
        with (
            tc.tile_pool(name="const", bufs=1) as cpool,
            tc.tile_pool(name="xp", bufs=xbufs) as xpool,
            tc.tile_pool(name="zp", bufs=zbufs) as zpool,
            tc.tile_pool(name="pzp", bufs=pzbufs, space="PSUM") as pzpool,
        ):
            c_t = cpool.tile([128, 15 * m2], BF16)
            d_t = cpool.tile([128, 6 * m2], BF16)
            nc.sync.dma_start(out=c_t[:], in_=c_d[:])
            nc.sync.dma_start(out=d_t[:], in_=d_d[:])

            def body():
                for g, b in blocks:
                    r0 = b * s
                    lo, hi = max(r0 - 2, 0), min(r0 + s + 2, h)
                    p0, rows = lo - (r0 - 2), hi - lo
                    x_t = xpool.tile([128, xw], BF16, tag="x")
                    if b == nb - 1:
                        nc.vector.memzero(x_t[:, :])
                    nc.sync.dma_start(
                        out=x_t[p0 : p0 + rows, 2 : 2 + w], in_=x_d[g, lo:hi, :]
                    )
                    if b == 0:
                        nc.vector.memzero(x_t[0:2, :])
                    if b != nb - 1:
                        nc.vector.memzero(x_t[:, 0:2])
                        nc.vector.memzero(x_t[:, 2 + w : 4 + w])
                    k = k_last if b == nb - 1 else 128
                    v = 0 if b == 0 else (2 if b == nb - 1 else 1)
                    rows_out = min(s, h - r0)
                    z_t = zpool.tile([m2, w], BF16, tag="z")
                    for j in range(nch):
                        pz = pzpool.tile([m2, nw], F32, tag="pz")
                        corrj = (j == 0) or (j == nch - 1)
                        for dx in range(5):
                            nc.tensor.matmul(
                                pz[:],
                                c_t[0:k, (v * 5 + dx) * m2 : (v * 5 + dx + 1) * m2],
                                x_t[0:k, nw * j + dx : nw * j + dx + nw],
                                start=(dx == 0),
                                stop=(dx == 4 and not corrj),
                            )
                        if corrj and j == 0:
                            nc.tensor.matmul(
                                pz[:, 0:2],
                                d_t[0:k, (v * 2 + 0) * m2 : (v * 2 + 1) * m2],
                                x_t[0:k, 2:0:-1],  # cols [x 0, zero]
                                start=False,
                                stop=(j != nch - 1),
                            )
                        if corrj and j == nch - 1:
                            nc.tensor.matmul(
                                pz[:, nw - 2 : nw],
                                d_t[0:k, (v * 2 + 1) * m2 : (v * 2 + 2) * m2],
                                x_t[0:k, w + 2 : w : -1],  # cols [zero, x w-1]
                                start=False,
                                stop=True,
                            )
                        if j % 2 == 0:
                            nc.scalar.copy(out=z_t[:, nw * j : nw * j + nw], in_=pz[:])
                        else:
                            nc.vector.tensor_copy(
                                out=z_t[:, nw * j : nw * j + nw], in_=pz[:]
                            )
                    nc.scalar.dma_start(
                        out=z_d[g, r0 : r0 + rows_out, :], in_=z_t[0:rows_out, :]
                    )

            if hwloop > 1:
                with tc.For_i(0, hwloop):
                    body()
            else:
                body()

    nc.compile()
    return nc


def build_nc8(imgs, h, w, nw=512, s=124, hwloop=1, xbufs=6, zbufs=4, pzbufs=8):
    """build_nc7 with the stationary band sections zero-padded from m2=124
    to 128 columns. A 128-column weight load triggers the PE's automatic
    Fast Weight Load path (2x for bf16); PSUM tiles grow to 128 partitions
    (rows 124..127 compute zeros) but still fit one 2KB bank."""
    assert w % nw == 0 and nw <= 512
    nb = -(-h // s)
    m2 = s
    ms = 128  # padded stationary columns / PSUM partitions
    nch = w // nw
    xw = w + 4
    r0_last = (nb - 1) * s
    rows_last = h - (r0_last - 2)
    k_last = rows_last + 2

    nc = bacc.Bacc("TRN2", target_bir_lowering=False, debug=False)
    x_d = nc.dram_tensor("x", [imgs, h, w], BF16, kind="ExternalInput")
    c_d = nc.dram_tensor("b1", [128, 15 * ms], BF16, kind="ExternalInput")
    d_d = nc.dram_tensor("b2", [128, 6 * ms], BF16, kind="ExternalInput")
    z_d = nc.dram_tensor("z", [imgs, h, w], BF16, kind="ExternalOutput")

    blocks = [(g, b) for g in range(imgs) for b in range(nb)]

    with tile.TileContext(nc) as tc:
        with (
            tc.tile_pool(name="const", bufs=1) as cpool,
            tc.tile_pool(name="xp", bufs=xbufs) as xpool,
            tc.tile_pool(name="zp", bufs=zbufs) as zpool,
            tc.tile_pool(name="pzp", bufs=pzbufs, space="PSUM") as pzpool,
        ):
            c_t = cpool.tile([128, 15 * ms], BF16)
            d_t = cpool.tile([128, 6 * ms], BF16)
            nc.sync.dma_start(out=c_t[:], in_=c_d[:])
            nc.sync.dma_start(out=d_t[:], in_=d_d[:])

            def body():
                for g, b in blocks:
                    r0 = b * s
                    lo, hi = max(r0 - 2, 0), min(r0 + s + 2, h)
                    p0, rows = lo - (r0 - 2), hi - lo
                    x_t = xpool.tile([128, xw], BF16, tag="x")
                    if b == nb - 1:
                        # only partitions >= rows hold stale data; compute-engine
                        # APs start at partition 0/32/64/96 and must not span
                        # more partitions than their alignment allows
                        zp0 = max(p for p in (0, 32, 64, 96) if p <= rows)
                        for q0, q1 in ((0, 32), (32, 64), (64, 128)):
                            if q1 > zp0:
                                nc.vector.memzero(x_t[max(q0, zp0) : q1, :])
                    nc.sync.dma_start(
                        out=x_t[p0 : p0 + rows, 2 : 2 + w], in_=x_d[g, lo:hi, :]
                    )
                    if b == 0:
                        nc.vector.memzero(x_t[0:2, :])
                    nc.vector.memzero(x_t[:, 0:2])
                    nc.vector.memzero(x_t[:, 2 + w : 4 + w])
                    k = k_last if b == nb - 1 else 128
                    v = 0 if b == 0 else (2 if b == nb - 1 else 1)
                    rows_out = min(s, h - r0)
                    z_t = zpool.tile([m2, w], BF16, tag="z")
                    for j in range(nch):
                        pz = pzpool.tile([ms, nw], F32, tag="pz")
                        corrj = (j == 0) or (j == nch - 1)
                        for dx in range(5):
                            nc.tensor.matmul(
                                pz[:],
                                c_t[0:k, (v * 5 + dx) * ms : (v * 5 + dx + 1) * ms],
                                x_t[0:k, nw * j + dx : nw * j + dx + nw],
                                start=(dx == 0),
                                stop=(dx == 4 and not corrj),
                            )
                        if corrj and j == 0:
                            nc.tensor.matmul(
                                pz[:, 0:2],
                                d_t[0:k, (v * 2 + 0) * ms : (v * 2 + 1) * ms],
                                x_t[0:k, 2:0:-1],  # cols [x 0, zero]
                                start=False,
                                stop=(j != nch - 1),
                            )
                        if corrj and j == nch - 1:
                            nc.tensor.matmul(
                                pz[:, nw - 2 : nw],
                                d_t[0:k, (v * 2 + 1) * ms : (v * 2 + 2) * ms],
                                x_t[0:k, w + 2 : w : -1],  # cols [zero, x w-1]
                                start=False,
                                stop=True,
                            )
                        if j % 2 == 0:
                            nc.scalar.copy(
                                out=z_t[:, nw * j : nw * j + nw], in_=pz[0:m2, :]
                            )
                        else:
                            nc.vector.tensor_copy(
                                out=z_t[:, nw * j : nw * j + nw], in_=pz[0:m2, :]
                            )
                    nc.scalar.dma_start(
                        out=z_d[g, r0 : r0 + rows_out, :], in_=z_t[0:rows_out, :]
                    )

            if hwloop > 1:
                with tc.For_i(0, hwloop):
                    body()
            else:
                body()

    nc.compile()
    return nc


def build_nc9(
    imgs,
    h,
    w,
    nw=512,
    s=124,
    hwloop=1,
    gT=4,
    xgbufs=3,
    xsbufs=2,
    zgbufs=3,
    zsbufs=2,
    pzbufs=8,
    dxouter=False,
):
    """build_nc8 with DMA batching: interior blocks' x loads are merged into
    one strided DMA per gT blocks (overlapping 4-row halos re-read from HBM)
    and z stores are merged per gT blocks, with DMA instructions alternating
    between the SP and ACT HW DGE rings. Measured per-instruction DMA cost
    (esp. the ~4us SBUF->HBM store completion) makes many small DMAs the
    bottleneck; merging + dual-ring cuts the DMA pass time under the PE time.
    """
    import concourse.bass as bass

    assert w % nw == 0 and nw <= 512
    nb = -(-h // s)
    m2 = s
    ms = 128
    nch = w // nw
    xw = w + 4
    r0_last = (nb - 1) * s
    rows_last = h - (r0_last - 2)
    k_last = rows_last + 2

    nc = bacc.Bacc("TRN2", target_bir_lowering=False, debug=False)
    x_d = nc.dram_tensor("x", [imgs, h, w], BF16, kind="ExternalInput")
    c_d = nc.dram_tensor("b1", [128, 15 * ms], BF16, kind="ExternalInput")
    d_d = nc.dram_tensor("b2", [128, 6 * ms], BF16, kind="ExternalInput")
    z_d = nc.dram_tensor("z", [imgs, h, w], BF16, kind="ExternalOutput")

    # load groups: block 0 and nb-1 single (need memzeroed halo rows); the
    # interior 1..nb-2 in chunks of gT. store groups: 0..nb-2 in chunks of
    # gT (uniform 124-row blocks), nb-1 single (ragged 64-row tail).
    ld_group = {}  # b -> (b0, T) for interior merged loads
    for b0 in range(1, nb - 1, gT):
        T = min(gT, nb - 1 - b0)
        for b in range(b0, b0 + T):
            ld_group[b] = (b0, T)
    st_group = {}  # b -> (b0, T)
    for b0 in range(0, nb - 1, gT):
        T = min(gT, nb - 1 - b0)
        for b in range(b0, b0 + T):
            st_group[b] = (b0, T)

    with tile.TileContext(nc) as tc:
        with (
            tc.tile_pool(name="const", bufs=1) as cpool,
            tc.tile_pool(name="xg", bufs=xgbufs) as xgpool,
            tc.tile_pool(name="xs", bufs=xsbufs) as xspool,
            tc.tile_pool(name="zg", bufs=zgbufs) as zgpool,
            tc.tile_pool(name="zs", bufs=zsbufs) as zspool,
            tc.tile_pool(name="pzp", bufs=pzbufs, space="PSUM") as pzpool,
        ):
            c_t = cpool.tile([128, 15 * ms], BF16)
            d_t = cpool.tile([128, 6 * ms], BF16)
            nc.sync.dma_start(out=c_t[:], in_=c_d[:])
            nc.sync.dma_start(out=d_t[:], in_=d_d[:])

            rings = [nc.sync, nc.scalar]
            ring_cnt = [0]

            def ring():
                e = rings[ring_cnt[0] & 1]
                ring_cnt[0] += 1
                return e

            def body():
                for g in range(imgs):
                    cur_x = None  # (b0, tile)
                    cur_z = None  # (b0, T, tile)
                    for b in range(nb):
                        r0 = b * s
                        # ---- x load ----
                        if b in ld_group:
                            b0, T = ld_group[b]
                            if b == b0:
                                xg = xgpool.tile([128, T, xw], BF16, tag="xg")
                                lo0 = b0 * s - 2
                                base = x_d[0, 0:1, 0:1]
                                src = bass.AP(
                                    tensor=base.tensor,
                                    offset=(g * h + lo0) * w,
                                    ap=[[w, 128], [s * w, T], [1, w]],
                                )
                                ring().dma_start(out=xg[:, :, 2 : 2 + w], in_=src)
                                nc.vector.memzero(xg[:, :, 0:2])
                                nc.vector.memzero(xg[:, :, 2 + w : 4 + w])
                                cur_x = (b0, xg)
                            b0x, xg = cur_x
                            x_t = xg[:, b - b0x, :]
                            k = 128
                        else:
                            lo, hi = max(r0 - 2, 0), min(r0 + s + 2, h)
                            p0, rows = lo - (r0 - 2), hi - lo
                            x_t = xspool.tile([128, xw], BF16, tag="xs")
                            if b == nb - 1:
                                zp0 = max(p for p in (0, 32, 64, 96) if p <= rows)
                                for q0, q1 in ((0, 32), (32, 64), (64, 128)):
                                    if q1 > zp0:
                                        nc.vector.memzero(x_t[max(q0, zp0) : q1, :])
                            ring().dma_start(
                                out=x_t[p0 : p0 + rows, 2 : 2 + w], in_=x_d[g, lo:hi, :]
                            )
                            if b == 0:
                                nc.vector.memzero(x_t[0:2, :])
                            nc.vector.memzero(x_t[:, 0:2])
                            nc.vector.memzero(x_t[:, 2 + w : 4 + w])
                            k = k_last if b == nb - 1 else 128
                        v = 0 if b == 0 else (2 if b == nb - 1 else 1)

                        # ---- z tile ----
                        if b in st_group:
                            b0z, Tz = st_group[b]
                            if b == b0z:
                                cur_z = (b0z, Tz, zgpool.tile([m2, Tz, w], BF16, tag="zg", name="zg"))
                            _, _, zg = cur_z
                            z_view = zg[:, b - b0z, :]
                        else:
                            z_view = zspool.tile([m2, w], BF16, tag="zs")

                        # ---- matmuls + copies ----
                        def copy_out(j, pz):
                            if j % 2 == 0:
                                nc.scalar.copy(
                                    out=z_view[:, nw * j : nw * j + nw], in_=pz[0:m2, :]
                                )
                            else:
                                nc.vector.tensor_copy(
                                    out=z_view[:, nw * j : nw * j + nw], in_=pz[0:m2, :]
                                )

                        if not dxouter:
                            for j in range(nch):
                                pz = pzpool.tile([ms, nw], F32, tag="pz")
                                corrj = (j == 0) or (j == nch - 1)
                                for dx in range(5):
                                    nc.tensor.matmul(
                                        pz[:],
                                        c_t[0:k, (v * 5 + dx) * ms : (v * 5 + dx + 1) * ms],
                                        x_t[0:k, nw * j + dx : nw * j + dx + nw],
                                        start=(dx == 0),
                                        stop=(dx == 4 and not corrj),
                                    )
                                if corrj and j == 0:
                                    nc.tensor.matmul(
                                        pz[:, 0:2],
                                        d_t[0:k, (v * 2 + 0) * ms : (v * 2 + 1) * ms],
                                        x_t[0:k, 2:0:-1],
                                        start=False,
                                        stop=(j != nch - 1),
                                    )
                                if corrj and j == nch - 1:
                                    nc.tensor.matmul(
                                        pz[:, nw - 2 : nw],
                                        d_t[0:k, (v * 2 + 1) * ms : (v * 2 + 2) * ms],
                                        x_t[0:k, w + 2 : w : -1],
                                        start=False,
                                        stop=True,
                                    )
                                copy_out(j, pz)
                        else:
                            pzs = [pzpool.tile([ms, nw], F32, tag="pz", name=f"pz{j}") for j in range(nch)]
                            for dx in range(5):
                                for j in range(nch):
                                    nc.tensor.matmul(
                                        pzs[j][:],
                                        c_t[0:k, (v * 5 + dx) * ms : (v * 5 + dx + 1) * ms],
                                        x_t[0:k, nw * j + dx : nw * j + dx + nw],
                                        start=(dx == 0),
                                        stop=(dx == 4 and j not in (0, nch - 1)),
                                    )
                            nc.tensor.matmul(
                                pzs[0][:, 0:2],
                                d_t[0:k, (v * 2 + 0) * ms : (v * 2 + 1) * ms],
                                x_t[0:k, 2:0:-1],
                                start=False,
                                stop=True,
                            )
                            nc.tensor.matmul(
                                pzs[nch - 1][:, nw - 2 : nw],
                                d_t[0:k, (v * 2 + 1) * ms : (v * 2 + 2) * ms],
                                x_t[0:k, w + 2 : w : -1],
                                start=False,
                                stop=True,
                            )
                            for j in range(nch):
                                copy_out(j, pzs[j])

                        # ---- z store ----
                        if b in st_group:
                            b0z, Tz, zg = cur_z
                            if b == b0z + Tz - 1:
                                zbase = z_d[0, 0:1, 0:1]
                                dst = bass.AP(
                                    tensor=zbase.tensor,
                                    offset=(g * h + b0z * s) * w,
                                    ap=[[w, m2], [s * w, Tz], [1, w]],
                                )
                                ring().dma_start(out=dst, in_=zg[:, :, :])
                        else:
                            rows_out = min(s, h - r0)
                            ring().dma_start(
                                out=z_d[g, r0 : r0 + rows_out, :],
                                in_=z_view[0:rows_out, :],
                            )

            if hwloop > 1:
                with tc.For_i(0, hwloop):
                    body()
            else:
                body()

    nc.compile()
    return nc


def build_nc10(imgs, h, w, nw=512, s=124, hwloop=1, xbufs=8, zbufs=6, pzbufs=8):
    """build_nc8 (per-block DMAs, good DRAM locality) with each block's x
    load and z store alternating between the SP and ACT HW DGE rings, so
    the ~4us per-store completion cost is paid on two rings in parallel."""
    assert w % nw == 0 and nw <= 512
    nb = -(-h // s)
    m2 = s
    ms = 128
    nch = w // nw
    xw = w + 4
    r0_last = (nb - 1) * s
    rows_last = h - (r0_last - 2)
    k_last = rows_last + 2

    nc = bacc.Bacc("TRN2", target_bir_lowering=False, debug=False)
    x_d = nc.dram_tensor("x", [imgs, h, w], BF16, kind="ExternalInput")
    c_d = nc.dram_tensor("b1", [128, 15 * ms], BF16, kind="ExternalInput")
    d_d = nc.dram_tensor("b2", [128, 6 * ms], BF16, kind="ExternalInput")
    z_d = nc.dram_tensor("z", [imgs, h, w], BF16, kind="ExternalOutput")

    blocks = [(g, b) for g in range(imgs) for b in range(nb)]

    with tile.TileContext(nc) as tc:
        with (
            tc.tile_pool(name="const", bufs=1) as cpool,
            tc.tile_pool(name="xp", bufs=xbufs) as xpool,
            tc.tile_pool(name="zp", bufs=zbufs) as zpool,
            tc.tile_pool(name="pzp", bufs=pzbufs, space="PSUM") as pzpool,
        ):
            c_t = cpool.tile([128, 15 * ms], BF16)
            d_t = cpool.tile([128, 6 * ms], BF16)
            nc.sync.dma_start(out=c_t[:], in_=c_d[:])
            nc.sync.dma_start(out=d_t[:], in_=d_d[:])

            def body():
                for t, (g, b) in enumerate(blocks):
                    r0 = b * s
                    lo, hi = max(r0 - 2, 0), min(r0 + s + 2, h)
                    p0, rows = lo - (r0 - 2), hi - lo
                    x_t = xpool.tile([128, xw], BF16, tag="x")
                    if b == nb - 1:
                        zp0 = max(p for p in (0, 32, 64, 96) if p <= rows)
                        for q0, q1 in ((0, 32), (32, 64), (64, 128)):
                            if q1 > zp0:
                                nc.vector.memzero(x_t[max(q0, zp0) : q1, :])
                    ld_eng = nc.sync if t % 2 == 0 else nc.scalar
                    st_eng = nc.scalar if t % 2 == 0 else nc.sync
                    ld_eng.dma_start(
                        out=x_t[p0 : p0 + rows, 2 : 2 + w], in_=x_d[g, lo:hi, :]
                    )
                    if b == 0:
                        nc.vector.memzero(x_t[0:2, :])
                    nc.vector.memzero(x_t[:, 0:2])
                    nc.vector.memzero(x_t[:, 2 + w : 4 + w])
                    k = k_last if b == nb - 1 else 128
                    v = 0 if b == 0 else (2 if b == nb - 1 else 1)
                    rows_out = min(s, h - r0)
                    z_t = zpool.tile([m2, w], BF16, tag="z")
                    for j in range(nch):
                        pz = pzpool.tile([ms, nw], F32, tag="pz")
                        corrj = (j == 0) or (j == nch - 1)
                        for dx in range(5):
                            nc.tensor.matmul(
                                pz[:],
                                c_t[0:k, (v * 5 + dx) * ms : (v * 5 + dx + 1) * ms],
                                x_t[0:k, nw * j + dx : nw * j + dx + nw],
                                start=(dx == 0),
                                stop=(dx == 4 and not corrj),
                            )
                        if corrj and j == 0:
                            nc.tensor.matmul(
                                pz[:, 0:2],
                                d_t[0:k, (v * 2 + 0) * ms : (v * 2 + 1) * ms],
                                x_t[0:k, 2:0:-1],
                                start=False,
                                stop=(j != nch - 1),
                            )
                        if corrj and j == nch - 1:
                            nc.tensor.matmul(
                                pz[:, nw - 2 : nw],
                                d_t[0:k, (v * 2 + 1) * ms : (v * 2 + 2) * ms],
                                x_t[0:k, w + 2 : w : -1],
                                start=False,
                                stop=True,
                            )
                        if j % 2 == 0:
                            nc.scalar.copy(
                                out=z_t[:, nw * j : nw * j + nw], in_=pz[0:m2, :]
                            )
                        else:
                            nc.vector.tensor_copy(
                                out=z_t[:, nw * j : nw * j + nw], in_=pz[0:m2, :]
                            )
                    st_eng.dma_start(
                        out=z_d[g, r0 : r0 + rows_out, :], in_=z_t[0:rows_out, :]
                    )

            if hwloop > 1:
                with tc.For_i(0, hwloop):
                    body()
            else:
                body()

    nc.compile()
    return nc


def build_nc11(
    imgs, h, w, nw=512, s=124, hwloop=1, xbufs=8, zbufs=6, pzbufs=8, gp_every=3
):
    """build_nc10 + every gp_every-th z store issued via the gpsimd SWDGE,
    adding a third independent DMA path (~37 GB/s) to the ~93 GB/s HWDGE
    store cap that paces the whole kernel."""
    assert w % nw == 0 and nw <= 512
    nb = -(-h // s)
    m2 = s
    ms = 128
    nch = w // nw
    xw = w + 4
    r0_last = (nb - 1) * s
    rows_last = h - (r0_last - 2)
    k_last = rows_last + 2

    nc = bacc.Bacc("TRN2", target_bir_lowering=False, debug=False)
    x_d = nc.dram_tensor("x", [imgs, h, w], BF16, kind="ExternalInput")
    c_d = nc.dram_tensor("b1", [128, 15 * ms], BF16, kind="ExternalInput")
    d_d = nc.dram_tensor("b2", [128, 6 * ms], BF16, kind="ExternalInput")
    z_d = nc.dram_tensor("z", [imgs, h, w], BF16, kind="ExternalOutput")

    blocks = [(g, b) for g in range(imgs) for b in range(nb)]

    with tile.TileContext(nc) as tc:
        with (
            tc.tile_pool(name="const", bufs=1) as cpool,
            tc.tile_pool(name="xp", bufs=xbufs) as xpool,
            tc.tile_pool(name="zp", bufs=zbufs) as zpool,
            tc.tile_pool(name="pzp", bufs=pzbufs, space="PSUM") as pzpool,
        ):
            c_t = cpool.tile([128, 15 * ms], BF16)
            d_t = cpool.tile([128, 6 * ms], BF16)
            nc.sync.dma_start(out=c_t[:], in_=c_d[:])
            nc.sync.dma_start(out=d_t[:], in_=d_d[:])

            def body():
                for t, (g, b) in enumerate(blocks):
                    r0 = b * s
                    lo, hi = max(r0 - 2, 0), min(r0 + s + 2, h)
                    p0, rows = lo - (r0 - 2), hi - lo
                    x_t = xpool.tile([128, xw], BF16, tag="x")
                    if b == nb - 1:
                        zp0 = max(p for p in (0, 32, 64, 96) if p <= rows)
                        for q0, q1 in ((0, 32), (32, 64), (64, 128)):
                            if q1 > zp0:
                                nc.vector.memzero(x_t[max(q0, zp0) : q1, :])
                    ld_eng = nc.sync if t % 2 == 0 else nc.scalar
                    ld_eng.dma_start(
                        out=x_t[p0 : p0 + rows, 2 : 2 + w], in_=x_d[g, lo:hi, :]
                    )
                    if b == 0:
                        nc.vector.memzero(x_t[0:2, :])
                    nc.vector.memzero(x_t[:, 0:2])
                    nc.vector.memzero(x_t[:, 2 + w : 4 + w])
                    k = k_last if b == nb - 1 else 128
                    v = 0 if b == 0 else (2 if b == nb - 1 else 1)
                    rows_out = min(s, h - r0)
                    z_t = zpool.tile([m2, w], BF16, tag="z")
                    for j in range(nch):
                        pz = pzpool.tile([ms, nw], F32, tag="pz")
                        corrj = (j == 0) or (j == nch - 1)
                        for dx in range(5):
                            nc.tensor.matmul(
                                pz[:],
                                c_t[0:k, (v * 5 + dx) * ms : (v * 5 + dx + 1) * ms],
                                x_t[0:k, nw * j + dx : nw * j + dx + nw],
                                start=(dx == 0),
                                stop=(dx == 4 and not corrj),
                            )
                        if corrj and j == 0:
                            nc.tensor.matmul(
                                pz[:, 0:2],
                                d_t[0:k, (v * 2 + 0) * ms : (v * 2 + 1) * ms],
                                x_t[0:k, 2:0:-1],
                                start=False,
                                stop=(j != nch - 1),
                            )
                        if corrj and j == nch - 1:
                            nc.tensor.matmul(
                                pz[:, nw - 2 : nw],
                                d_t[0:k, (v * 2 + 1) * ms : (v * 2 + 2) * ms],
                                x_t[0:k, w + 2 : w : -1],
                                start=False,
                                stop=True,
                            )
                        if j % 2 == 0:
                            nc.scalar.copy(
                                out=z_t[:, nw * j : nw * j + nw], in_=pz[0:m2, :]
                            )
                        else:
                            nc.vector.tensor_copy(
                                out=z_t[:, nw * j : nw * j + nw], in_=pz[0:m2, :]
                            )
                    if gp_every and t % gp_every == gp_every - 1:
                        st_eng = nc.gpsimd
                    else:
                        st_eng = nc.scalar if t % 2 == 0 else nc.sync
                    st_eng.dma_start(
                        out=z_d[g, r0 : r0 + rows_out, :], in_=z_t[0:rows_out, :]
                    )

            if hwloop > 1:
                with tc.For_i(0, hwloop):
                    body()
            else:
                body()

    nc.compile()
    return nc


def _pad_bands_128(b1, b2, m2):
    c = b1.reshape(128, 15, m2)
    cp = np.zeros((128, 15, 128), b1.dtype)
    cp[:, :, :m2] = c
    d = b2.reshape(128, 6, m2)
    dp = np.zeros((128, 6, 128), b2.dtype)
    dp[:, :, :m2] = d
    return (
        np.ascontiguousarray(cp.reshape(128, 15 * 128)),
        np.ascontiguousarray(dp.reshape(128, 6 * 128)),
    )


def to_bf16(a):
    import ml_dtypes

    return np.ascontiguousarray(np.asarray(a).astype(ml_dtypes.bfloat16))


def make_in_maps(x, w1, w2, h=FULL_H, s=124):
    """bf16 per-core input maps for build_nc8 from full f32 inputs."""
    nb = -(-h // s)
    b1, b2 = _build_bands5(w1, w2, h, s, nb)
    b1, b2 = _pad_bands_128(b1, b2, s)
    b1, b2 = to_bf16(b1), to_bf16(b2)
    xb = to_bf16(np.asarray(x, np.float32).reshape(FULL_B, FULL_H, FULL_W))
    imgs = FULL_B // NCORES
    return [
        {"x": np.ascontiguousarray(xb[imgs * c : imgs * (c + 1)]), "b1": b1, "b2": b2}
        for c in range(NCORES)
    ]


_NC_CACHE = None


def kernel(x, w1, w2):
    global _NC_CACHE, LAST_RESULTS
    in_maps = make_in_maps(x, w1, w2)
    if _NC_CACHE is None:
        _NC_CACHE = build_nc11(FULL_B // NCORES, FULL_H, FULL_W, nw=512, s=124)
    nc = _NC_CACHE
    res = run_bass_kernel_spmd(nc, in_maps, core_ids=list(range(NCORES)), trace=TRACE)
    LAST_RESULTS = res
    out = np.stack(
        [np.asarray(res.results[c]["z"], np.float32) for c in range(NCORES)], axis=0
    )
    return out.reshape(FULL_B, 1, FULL_H, FULL_W)



# revision 18
# speedup vs baseline: 1.2715x; 1.0795x over previous
"""Trainium2 Bass kernel: two chained SAME-padded 3x3 single-channel convs.

  reference: z = conv3x3(conv3x3(x, w1), w2)   x: [16,1,2048,2048] f32

Strategy (pure data parallel, 2 images per core on 8 cores):
  - Images processed in horizontal bands of S=124 output rows.
  - Each conv is computed on the TensorEngine as 3 banded matmuls
    (one per kernel column dx), accumulating in PSUM. The band matrix
    B_dx[k, m] = W[k-m, dx] applies the vertical taps; the horizontal
    taps come from shifting the moving operand (rhs) by dx columns.
  - conv1 output (y) is copied PSUM->SBUF (VectorE) and consumed by
    conv2's matmuls; conv2 output is copied PSUM->SBUF (ScalarE) and
    DMA'd out. Intermediate y never touches HBM.
  - float32r matmuls (1 cycle/row on the PE vs 4 for fp32; ~1.5e-4
    absmax-relative rounding measured on HW). Set MM_DT = F32 below for
    full fp32 precision at ~4x the PE cost.
  - SAME padding handled with zeroed halo columns in SBUF and
    host-built band-matrix variants for the top/bottom image edges.

Band matrices are built on the host from w1/w2 (they are just 9 floats
each) and passed as extra inputs.
"""

import hashlib
import os
import shutil

import numpy as np

import concourse.mybir as mybir
import concourse.tile as tile
from concourse import bacc, bass2jax
from concourse.bass_utils import run_bass_kernel_spmd


def _install_neff_disk_cache():
    """Cache compiled NEFFs on disk keyed by BIR content hash — the
    neuronxcc backend takes minutes for this kernel and has no cache of
    its own, so a fresh process would otherwise recompile every run."""
    if getattr(bass2jax, "_ant_neff_cache_installed", False):
        return
    orig = bass2jax.compile_bir_kernel

    def cached(bir_json, tmpdir, neff_name="file.neff"):
        try:
            cdir = os.path.expanduser("~/.cache/bass_neff")
            os.makedirs(cdir, exist_ok=True)
            key = hashlib.sha256(
                bir_json if isinstance(bir_json, bytes) else bir_json.encode()
            ).hexdigest()[:32]
            cpath = os.path.join(cdir, f"{key}.neff")
            if os.path.exists(cpath):
                outdir = os.path.join(tmpdir, "sg00")
                os.makedirs(outdir, exist_ok=True)
                dst = os.path.join(outdir, neff_name)
                shutil.copyfile(cpath, dst)
                return dst
            neff = orig(bir_json, tmpdir, neff_name)
            shutil.copyfile(neff, cpath + ".tmp")
            os.replace(cpath + ".tmp", cpath)
            return neff
        except Exception:
            return orig(bir_json, tmpdir, neff_name)

    bass2jax.compile_bir_kernel = cached
    bass2jax._ant_neff_cache_installed = True


_install_neff_disk_cache()

F32 = mybir.dt.float32
F32R = mybir.dt.float32r

MM_DT = F32R  # matmul operand dtype: F32R (fast) or F32 (exact)

NCORES = 8
FULL_B, FULL_H, FULL_W = 16, 2048, 2048

TRACE = False  # set True (from test harness) to capture an NTFF profile
LAST_RESULTS = None  # BassKernelResults of the most recent run


def _build_bands(w1, w2, h, s, nb):
    """Host-side band matrices for the vertical taps.

    B1: [128, 3 variants, 3 dx, 126]; variant 0 = top block, 1 = mid,
    2 = bottom block.  B2: [126, 3 dx, 124].
    """
    W1 = np.asarray(w1, np.float32).reshape(3, 3)
    W2 = np.asarray(w2, np.float32).reshape(3, 3)
    m1, m2 = s + 2, s
    b1 = np.zeros((128, 3, 3, m1), np.float32)
    b2 = np.zeros((m1, 3, m2), np.float32)
    mm = np.arange(m1)
    for i in range(3):
        for dx in range(3):
            b1[mm + i, :, dx, mm] = W1[i, dx]
    mm = np.arange(m2)
    for i in range(3):
        for dx in range(3):
            b2[mm + i, dx, mm] = W2[i, dx]
    # top block: y row r0-1 = -1 is conv2's zero padding, not a computed row
    b1[:, 0, :, 0] = 0.0
    # bottom block: y row == h is zero padding
    r0_last = (nb - 1) * s
    b1[:, 2, :, h - r0_last + 1] = 0.0
    return np.ascontiguousarray(b1.reshape(128, 9 * m1)), np.ascontiguousarray(
        b2.reshape(m1, 3 * m2)
    )


def build_nc(imgs, h, w, nw=512, s=124, repeat=1):
    """Build the per-core Bass program (parametric so a small config can
    be validated in CoreSim)."""
    assert w % nw == 0 and nw <= 512
    nb = -(-h // s)  # blocks per image
    m1, m2 = s + 2, s  # conv1/conv2 output rows per block
    nch = w // nw  # width chunks
    xw = w + 4  # tile width: col 0 zero | 1..w data | w+1 zero | pad
    r0_last = (nb - 1) * s
    rows_last = h - (r0_last - 2)  # x rows loaded for the last block
    k1_last = rows_last + 2

    nc = bacc.Bacc("TRN2", target_bir_lowering=False, debug=False)
    x_d = nc.dram_tensor("x", [imgs, h, w], MM_DT, kind="ExternalInput")
    b1_d = nc.dram_tensor("b1", [128, 9 * m1], MM_DT, kind="ExternalInput")
    b2_d = nc.dram_tensor("b2", [m1, 3 * m2], MM_DT, kind="ExternalInput")
    z_d = nc.dram_tensor("z", [imgs, h, w], F32, kind="ExternalOutput")

    blocks = [(g, b) for g in range(imgs) for b in range(nb)] * repeat

    with tile.TileContext(nc) as tc:
        with (
            tc.tile_pool(name="const", bufs=1) as cpool,
            tc.tile_pool(name="xp", bufs=3) as xpool,
            tc.tile_pool(name="yp", bufs=2) as ypool,
            tc.tile_pool(name="zp", bufs=2) as zpool,
            tc.tile_pool(name="pyp", bufs=4, space="PSUM") as pypool,
            tc.tile_pool(name="pzp", bufs=4, space="PSUM") as pzpool,
        ):
            b1_t = cpool.tile([128, 9 * m1], MM_DT)
            b2_t = cpool.tile([m1, 3 * m2], MM_DT)
            nc.sync.dma_start(out=b1_t[:], in_=b1_d[:])
            nc.sync.dma_start(out=b2_t[:], in_=b2_d[:])

            pend = None  # (img, block, y_tile) awaiting conv2
            for t in range(len(blocks) + 1):
                if t < len(blocks):
                    g, b = blocks[t]
                    r0 = b * s
                    lo, hi = max(r0 - 2, 0), min(r0 + m1, h)
                    p0, rows = lo - (r0 - 2), hi - lo
                    x_t = xpool.tile([128, xw], MM_DT, tag="x")
                    if b == nb - 1:
                        # bottom block: zero the whole tile first (covers the
                        # 2 halo rows below the image and the halo columns);
                        # compute-engine APs can only start at partition
                        # 0/32/64/96, so a targeted halo-row memset is not
                        # expressible.
                        nc.vector.memzero(x_t[:, :])
                    nc.sync.dma_start(
                        out=x_t[p0 : p0 + rows, 1 : 1 + w], in_=x_d[g, lo:hi, :]
                    )
                    if b == 0:
                        nc.vector.memzero(x_t[0:2, :])
                    if b != nb - 1:
                        nc.vector.memzero(x_t[:, 0:1])
                        nc.vector.memzero(x_t[:, 1 + w : 2 + w])
                    k1 = k1_last if b == nb - 1 else 128
                    v = 0 if b == 0 else (2 if b == nb - 1 else 1)
                    y_t = ypool.tile([m1, xw], MM_DT, tag="y")
                    for j in range(nch):
                        py = pypool.tile([m1, nw], F32, tag="py")
                        for dx in range(3):
                            nc.tensor.matmul(
                                py[:],
                                b1_t[0:k1, (v * 3 + dx) * m1 : (v * 3 + dx + 1) * m1],
                                x_t[0:k1, nw * j + dx : nw * j + dx + nw],
                                start=(dx == 0),
                                stop=(dx == 2),
                            )
                        nc.vector.tensor_copy(
                            out=y_t[:, 1 + nw * j : 1 + nw * j + nw], in_=py[:]
                        )
                    nc.vector.memzero(y_t[:, 0:1])
                    nc.vector.memzero(y_t[:, 1 + w : 2 + w])
                    pend_next = (g, b, y_t)
                else:
                    pend_next = None

                if pend is not None:
                    g2, b2i, y_prev = pend
                    r0 = b2i * s
                    rows = min(s, h - r0)
                    z_t = zpool.tile([m2, w], F32, tag="z")
                    for j in range(nch):
                        pz = pzpool.tile([m2, nw], F32, tag="pz")
                        for dx in range(3):
                            nc.tensor.matmul(
                                pz[:],
                                b2_t[0:m1, dx * m2 : (dx + 1) * m2],
                                y_prev[0:m1, nw * j + dx : nw * j + dx + nw],
                                start=(dx == 0),
                                stop=(dx == 2),
                            )
                        nc.scalar.copy(out=z_t[:, nw * j : nw * j + nw], in_=pz[:])
                    nc.sync.dma_start(out=z_d[g2, r0 : r0 + rows, :], in_=z_t[0:rows, :])
                pend = pend_next

    nc.compile()
    return nc


def _build_bands5(w1, w2, h, s, nb):
    """Composite single-pass operator: z = C(x) where C = conv2 o conv1
    with the chained-SAME-padding semantics folded in exactly.

    Vertical behavior (including the y[-1]/y[h] zero rows and the image
    top/bottom) is encoded in per-variant 5-diagonal band matrices
    C[k, v, dx, m].  The only horizontal discrepancy of the composite
    vs the chained convs is the phantom y column at each side; D holds
    the two exact correction bands (applied to x col 0 / w-1, adding
    into z col 0 / w-1).
    """
    W1 = np.asarray(w1, np.float64).reshape(3, 3)
    W2 = np.asarray(w2, np.float64).reshape(3, 3)
    m1, m2 = s + 2, s

    def a_mat(col, rows, cols):
        a = np.zeros((rows, cols), np.float64)
        r = np.arange(rows)
        for i in range(3):
            a[r, r + i] = col[i]
        return a

    r0_last = (nb - 1) * s
    c = np.zeros((128, 3, 5, m2), np.float64)
    d = np.zeros((128, 3, 2, m2), np.float64)
    for v in range(3):
        a1 = [a_mat(W1[:, j], m1, 128) for j in range(3)]
        if v == 0:
            for a in a1:
                a[0, :] = 0.0  # y row -1 is conv2 zero padding
        if v == 2:
            for a in a1:
                a[h - r0_last + 1, :] = 0.0  # y row h is zero padding
        a2 = [a_mat(W2[:, j], m2, m1) for j in range(3)]
        for j in range(3):
            for jp in range(3):
                c[:, v, j + jp, :] += (a2[jp] @ a1[j]).T
        d[:, v, 0, :] = -(a2[0] @ a1[2]).T
        d[:, v, 1, :] = -(a2[2] @ a1[0]).T
    return (
        np.ascontiguousarray(c.reshape(128, 15 * m2).astype(np.float32)),
        np.ascontiguousarray(d.reshape(128, 6 * m2).astype(np.float32)),
    )


def build_nc5(imgs, h, w, nw=512, s=124, repeat=1, xbufs=4, zbufs=3, pzbufs=8, zdma="sync", corr=True):
    """Single-pass composite-5x5 program (see _build_bands5)."""
    assert w % nw == 0 and nw <= 512
    nb = -(-h // s)
    m2 = s
    nch = w // nw
    xw = w + 4  # cols 0,1 zero | 2..w+1 data | w+2,w+3 zero
    r0_last = (nb - 1) * s
    rows_last = h - (r0_last - 2)
    k_last = rows_last + 2

    nc = bacc.Bacc("TRN2", target_bir_lowering=False, debug=False)
    x_d = nc.dram_tensor("x", [imgs, h, w], MM_DT, kind="ExternalInput")
    c_d = nc.dram_tensor("b1", [128, 15 * m2], MM_DT, kind="ExternalInput")
    d_d = nc.dram_tensor("b2", [128, 6 * m2], MM_DT, kind="ExternalInput")
    z_d = nc.dram_tensor("z", [imgs, h, w], F32, kind="ExternalOutput")

    blocks = [(g, b) for g in range(imgs) for b in range(nb)] * repeat

    with tile.TileContext(nc) as tc:
        with (
            tc.tile_pool(name="const", bufs=1) as cpool,
            tc.tile_pool(name="xp", bufs=xbufs) as xpool,
            tc.tile_pool(name="zp", bufs=zbufs) as zpool,
            tc.tile_pool(name="pzp", bufs=pzbufs, space="PSUM") as pzpool,
        ):
            c_t = cpool.tile([128, 15 * m2], MM_DT)
            d_t = cpool.tile([128, 6 * m2], MM_DT)
            nc.sync.dma_start(out=c_t[:], in_=c_d[:])
            nc.sync.dma_start(out=d_t[:], in_=d_d[:])

            for g, b in blocks:
                r0 = b * s
                lo, hi = max(r0 - 2, 0), min(r0 + s + 2, h)
                p0, rows = lo - (r0 - 2), hi - lo
                x_t = xpool.tile([128, xw], MM_DT, tag="x")
                if b == nb - 1:
                    nc.vector.memzero(x_t[:, :])
                nc.sync.dma_start(
                    out=x_t[p0 : p0 + rows, 2 : 2 + w], in_=x_d[g, lo:hi, :]
                )
                if b == 0:
                    nc.vector.memzero(x_t[0:2, :])
                if b != nb - 1:
                    nc.vector.memzero(x_t[:, 0:2])
                    nc.vector.memzero(x_t[:, 2 + w : 4 + w])
                k = k_last if b == nb - 1 else 128
                v = 0 if b == 0 else (2 if b == nb - 1 else 1)
                rows_out = min(s, h - r0)
                z_t = zpool.tile([m2, w], F32, tag="z")
                for j in range(nch):
                    pz = pzpool.tile([m2, nw], F32, tag="pz")
                    corrj = corr and ((j == 0) or (j == nch - 1))
                    for dx in range(5):
                        nc.tensor.matmul(
                            pz[:],
                            c_t[0:k, (v * 5 + dx) * m2 : (v * 5 + dx + 1) * m2],
                            x_t[0:k, nw * j + dx : nw * j + dx + nw],
                            start=(dx == 0),
                            stop=(dx == 4 and not corrj),
                        )
                    # fp32r matmuls need an even moving-operand count and an
                    # 8B-aligned even-count dst, so the 1-column corrections
                    # run as N=2 with the partner column reading a zeroed
                    # halo column of x (negative-step AP) -> contributes 0.
                    if corrj and j == 0:
                        nc.tensor.matmul(
                            pz[:, 0:2],
                            d_t[0:k, (v * 2 + 0) * m2 : (v * 2 + 1) * m2],
                            x_t[0:k, 2:0:-1],  # cols [x 0, zero]
                            start=False,
                            stop=(j != nch - 1),
                        )
                    if corrj and j == nch - 1:
                        nc.tensor.matmul(
                            pz[:, nw - 2 : nw],
                            d_t[0:k, (v * 2 + 1) * m2 : (v * 2 + 2) * m2],
                            x_t[0:k, w + 2 : w : -1],  # cols [zero, x w-1]
                            start=False,
                            stop=True,
                        )
                    if j % 2 == 0:
                        nc.scalar.copy(out=z_t[:, nw * j : nw * j + nw], in_=pz[:])
                    else:
                        nc.vector.tensor_copy(
                            out=z_t[:, nw * j : nw * j + nw], in_=pz[:]
                        )
                zeng = nc.scalar if zdma == "scalar" else nc.sync
                zeng.dma_start(
                    out=z_d[g, r0 : r0 + rows_out, :], in_=z_t[0:rows_out, :]
                )

    nc.compile()
    return nc


def build_nc6(
    imgs, h, w, nw=512, s=124, repeat=1, xbufs=4, zbufs=3, pzbufs=8, zdma="scalar"
):
    """Composite single-pass program over HOST-PADDED x.

    x arrives as [imgs, h+4, w+4] with 2 zero rows/cols on every side,
    so the device needs no halo memsets at all: each band of s output
    rows is one clean [<=128, w+4] DMA, 22 accumulating fp32r matmuls,
    4 PSUM->SBUF copies and one store."""
    assert w % nw == 0 and nw <= 512
    nb = -(-h // s)
    m2 = s
    nch = w // nw
    xw = w + 4
    r0_last = (nb - 1) * s
    k_last = h + 4 - r0_last  # padded rows available for the last block

    nc = bacc.Bacc("TRN2", target_bir_lowering=False, debug=False)
    x_d = nc.dram_tensor("x", [imgs, h + 4, w + 4], MM_DT, kind="ExternalInput")
    c_d = nc.dram_tensor("b1", [128, 15 * m2], MM_DT, kind="ExternalInput")
    d_d = nc.dram_tensor("b2", [128, 6 * m2], MM_DT, kind="ExternalInput")
    z_d = nc.dram_tensor("z", [imgs, h, w], F32, kind="ExternalOutput")

    blocks = [(g, b) for g in range(imgs) for b in range(nb)] * repeat

    with tile.TileContext(nc) as tc:
        with (
            tc.tile_pool(name="const", bufs=1) as cpool,
            tc.tile_pool(name="xp", bufs=xbufs) as xpool,
            tc.tile_pool(name="zp", bufs=zbufs) as zpool,
            tc.tile_pool(name="pzp", bufs=pzbufs, space="PSUM") as pzpool,
        ):
            c_t = cpool.tile([128, 15 * m2], MM_DT)
            d_t = cpool.tile([128, 6 * m2], MM_DT)
            nc.sync.dma_start(out=c_t[:], in_=c_d[:])
            nc.sync.dma_start(out=d_t[:], in_=d_d[:])

            for g, b in blocks:
                r0 = b * s
                k = k_last if b == nb - 1 else 128
                x_t = xpool.tile([128, xw], MM_DT, tag="x")
                nc.sync.dma_start(out=x_t[0:k, :], in_=x_d[g, r0 : r0 + k, :])
                v = 0 if b == 0 else (2 if b == nb - 1 else 1)
                rows_out = min(s, h - r0)
                z_t = zpool.tile([m2, w], F32, tag="z")
                for j in range(nch):
                    pz = pzpool.tile([m2, nw], F32, tag="pz")
                    corrj = (j == 0) or (j == nch - 1)
                    for dx in range(5):
                        nc.tensor.matmul(
                            pz[:],
                            c_t[0:k, (v * 5 + dx) * m2 : (v * 5 + dx + 1) * m2],
                            x_t[0:k, nw * j + dx : nw * j + dx + nw],
                            start=(dx == 0),
                            stop=(dx == 4 and not corrj),
                        )
                    if j == 0:
                        nc.tensor.matmul(
                            pz[:, 0:2],
                            d_t[0:k, (v * 2 + 0) * m2 : (v * 2 + 1) * m2],
                            x_t[0:k, 2:0:-1],  # cols [x 0, zero]
                            start=False,
                            stop=(j != nch - 1),
                        )
                    if j == nch - 1:
                        nc.tensor.matmul(
                            pz[:, nw - 2 : nw],
                            d_t[0:k, (v * 2 + 1) * m2 : (v * 2 + 2) * m2],
                            x_t[0:k, w + 2 : w : -1],  # cols [zero, x w-1]
                            start=False,
                            stop=True,
                        )
                    if j % 2 == 0:
                        nc.scalar.copy(out=z_t[:, nw * j : nw * j + nw], in_=pz[:])
                    else:
                        nc.vector.tensor_copy(
                            out=z_t[:, nw * j : nw * j + nw], in_=pz[:]
                        )
                zeng = nc.scalar if zdma == "scalar" else nc.sync
                zeng.dma_start(
                    out=z_d[g, r0 : r0 + rows_out, :], in_=z_t[0:rows_out, :]
                )

    nc.compile()
    return nc


def pad_x(x, imgs, h, w):
    xp = np.zeros((imgs, h + 4, w + 4), np.float32)
    xp[:, 2 : h + 2, 2 : w + 2] = x
    return xp


BF16 = mybir.dt.bfloat16


def build_nc7(imgs, h, w, nw=512, s=124, hwloop=1, xbufs=6, zbufs=4, pzbufs=8):
    """bf16 single-pass composite-5x5 program (see _build_bands5).

    vs build_nc5: x/z/bands in bf16 (halves HBM traffic; PSUM accumulation
    stays f32), z stores issued from the Activation engine so loads (SP
    ring) and stores (ACT ring) use the two independent HW DGE queues, and
    an optional hardware loop (`hwloop` passes per NEFF execution) for
    dispatch-overhead-free steady-state timing.
    """
    assert w % nw == 0 and nw <= 512
    nb = -(-h // s)
    m2 = s
    nch = w // nw
    xw = w + 4  # cols 0,1 zero | 2..w+1 data | w+2,w+3 zero
    r0_last = (nb - 1) * s
    rows_last = h - (r0_last - 2)
    k_last = rows_last + 2

    nc = bacc.Bacc("TRN2", target_bir_lowering=False, debug=False)
    x_d = nc.dram_tensor("x", [imgs, h, w], BF16, kind="ExternalInput")
    c_d = nc.dram_tensor("b1", [128, 15 * m2], BF16, kind="ExternalInput")
    d_d = nc.dram_tensor("b2", [128, 6 * m2], BF16, kind="ExternalInput")
    z_d = nc.dram_tensor("z", [imgs, h, w], BF16, kind="ExternalOutput")

    blocks = [(g, b) for g in range(imgs) for b in range(nb)]

    with tile.TileContext(nc) as tc:
        with (
            tc.tile_pool(name="const", bufs=1) as cpool,
            tc.tile_pool(name="xp", bufs=xbufs) as xpool,
            tc.tile_pool(name="zp", bufs=zbufs) as zpool,
            tc.tile_pool(name="pzp", bufs=pzbufs, space="PSUM") as pzpool,
        ):
            c_t = cpool.tile([128, 15 * m2], BF16)
            d_t = cpool.tile([128, 6 * m2], BF16)
            nc.sync.dma_start(out=c_t[:], in_=c_d[:])
            nc.sync.dma_start(out=d_t[:], in_=d_d[:])

            def body():
                for g, b in blocks:
                    r0 = b * s
                    lo, hi = max(r0 - 2, 0), min(r0 + s + 2, h)
                    p0, rows = lo - (r0 - 2), hi - lo
                    x_t = xpool.tile([128, xw], BF16, tag="x")
                    if b == nb - 1:
                        nc.vector.memzero(x_t[:, :])
                    nc.sync.dma_start(
                        out=x_t[p0 : p0 + rows, 2 : 2 + w], in_=x_d[g, lo:hi, :]
                    )
                    if b == 0:
                        nc.vector.memzero(x_t[0:2, :])
                    if b != nb - 1:
                        nc.vector.memzero(x_t[:, 0:2])
                        nc.vector.memzero(x_t[:, 2 + w : 4 + w])
                    k = k_last if b == nb - 1 else 128
                    v = 0 if b == 0 else (2 if b == nb - 1 else 1)
                    rows_out = min(s, h - r0)
                    z_t = zpool.tile([m2, w], BF16, tag="z")
                    for j in range(nch):
                        pz = pzpool.tile([m2, nw], F32, tag="pz")
                        corrj = (j == 0) or (j == nch - 1)
                        for dx in range(5):
                            nc.tensor.matmul(
                                pz[:],
                                c_t[0:k, (v * 5 + dx) * m2 : (v * 5 + dx + 1) * m2],
                                x_t[0:k, nw * j + dx : nw * j + dx + nw],
                                start=(dx == 0),
                                stop=(dx == 4 and not corrj),
                            )
                        if corrj and j == 0:
                            nc.tensor.matmul(
                                pz[:, 0:2],
                                d_t[0:k, (v * 2 + 0) * m2 : (v * 2 + 1) * m2],
                                x_t[0:k, 2:0:-1],  # cols [x 0, zero]
                                start=False,
                                stop=(j != nch - 1),
                            )
                        if corrj and j == nch - 1:
                            nc.tensor.matmul(
                                pz[:, nw - 2 : nw],
                                d_t[0:k, (v * 2 + 1) * m2 : (v * 2 + 2) * m2],
                                x_t[0:k, w + 2 : w : -1],  # cols [zero, x w-1]
                                start=False,
                                stop=True,
                            )
                        if j % 2 == 0:
                            nc.scalar.copy(out=z_t[:, nw * j : nw * j + nw], in_=pz[:])
                        else:
                            nc.vector.tensor_copy(
                                out=z_t[:, nw * j : nw * j + nw], in_=pz[:]
                            )
                    nc.scalar.dma_start(
                        out=z_d[g, r0 : r0 + rows_out, :], in_=z_t[0:rows_out, :]
                    )

            if hwloop > 1:
                with tc.For_i(0, hwloop):
                    body()
            else:
                body()

    nc.compile()
    return nc


def build_nc8(imgs, h, w, nw=512, s=124, hwloop=1, xbufs=6, zbufs=4, pzbufs=8):
    """build_nc7 with the stationary band sections zero-padded from m2=124
    to 128 columns. A 128-column weight load triggers the PE's automatic
    Fast Weight Load path (2x for bf16); PSUM tiles grow to 128 partitions
    (rows 124..127 compute zeros) but still fit one 2KB bank."""
    assert w % nw == 0 and nw <= 512
    nb = -(-h // s)
    m2 = s
    ms = 128  # padded stationary columns / PSUM partitions
    nch = w // nw
    xw = w + 4
    r0_last = (nb - 1) * s
    rows_last = h - (r0_last - 2)
    k_last = rows_last + 2

    nc = bacc.Bacc("TRN2", target_bir_lowering=False, debug=False)
    x_d = nc.dram_tensor("x", [imgs, h, w], BF16, kind="ExternalInput")
    c_d = nc.dram_tensor("b1", [128, 15 * ms], BF16, kind="ExternalInput")
    d_d = nc.dram_tensor("b2", [128, 6 * ms], BF16, kind="ExternalInput")
    z_d = nc.dram_tensor("z", [imgs, h, w], BF16, kind="ExternalOutput")

    blocks = [(g, b) for g in range(imgs) for b in range(nb)]

    with tile.TileContext(nc) as tc:
        with (
            tc.tile_pool(name="const", bufs=1) as cpool,
            tc.tile_pool(name="xp", bufs=xbufs) as xpool,
            tc.tile_pool(name="zp", bufs=zbufs) as zpool,
            tc.tile_pool(name="pzp", bufs=pzbufs, space="PSUM") as pzpool,
        ):
            c_t = cpool.tile([128, 15 * ms], BF16)
            d_t = cpool.tile([128, 6 * ms], BF16)
            nc.sync.dma_start(out=c_t[:], in_=c_d[:])
            nc.sync.dma_start(out=d_t[:], in_=d_d[:])

            def body():
                for g, b in blocks:
                    r0 = b * s
                    lo, hi = max(r0 - 2, 0), min(r0 + s + 2, h)
                    p0, rows = lo - (r0 - 2), hi - lo
                    x_t = xpool.tile([128, xw], BF16, tag="x")
                    if b == nb - 1:
                        # only partitions >= rows hold stale data; compute-engine
                        # APs start at partition 0/32/64/96 and must not span
                        # more partitions than their alignment allows
                        zp0 = max(p for p in (0, 32, 64, 96) if p <= rows)
                        for q0, q1 in ((0, 32), (32, 64), (64, 128)):
                            if q1 > zp0:
                                nc.vector.memzero(x_t[max(q0, zp0) : q1, :])
                    nc.sync.dma_start(
                        out=x_t[p0 : p0 + rows, 2 : 2 + w], in_=x_d[g, lo:hi, :]
                    )
                    if b == 0:
                        nc.vector.memzero(x_t[0:2, :])
                    nc.vector.memzero(x_t[:, 0:2])
                    nc.vector.memzero(x_t[:, 2 + w : 4 + w])
                    k = k_last if b == nb - 1 else 128
                    v = 0 if b == 0 else (2 if b == nb - 1 else 1)
                    rows_out = min(s, h - r0)
                    z_t = zpool.tile([m2, w], BF16, tag="z")
                    for j in range(nch):
                        pz = pzpool.tile([ms, nw], F32, tag="pz")
                        corrj = (j == 0) or (j == nch - 1)
                        for dx in range(5):
                            nc.tensor.matmul(
                                pz[:],
                                c_t[0:k, (v * 5 + dx) * ms : (v * 5 + dx + 1) * ms],
                                x_t[0:k, nw * j + dx : nw * j + dx + nw],
                                start=(dx == 0),
                                stop=(dx == 4 and not corrj),
                            )
                        if corrj and j == 0:
                            nc.tensor.matmul(
                                pz[:, 0:2],
                                d_t[0:k, (v * 2 + 0) * ms : (v * 2 + 1) * ms],
                                x_t[0:k, 2:0:-1],  # cols [x 0, zero]
                                start=False,
                                stop=(j != nch - 1),
                            )
                        if corrj and j == nch - 1:
                            nc.tensor.matmul(
                                pz[:, nw - 2 : nw],
                                d_t[0:k, (v * 2 + 1) * ms : (v * 2 + 2) * ms],
                                x_t[0:k, w + 2 : w : -1],  # cols [zero, x w-1]
                                start=False,
                                stop=True,
                            )
                        if j % 2 == 0:
                            nc.scalar.copy(
                                out=z_t[:, nw * j : nw * j + nw], in_=pz[0:m2, :]
                            )
                        else:
                            nc.vector.tensor_copy(
                                out=z_t[:, nw * j : nw * j + nw], in_=pz[0:m2, :]
                            )
                    nc.scalar.dma_start(
                        out=z_d[g, r0 : r0 + rows_out, :], in_=z_t[0:rows_out, :]
                    )

            if hwloop > 1:
                with tc.For_i(0, hwloop):
                    body()
            else:
                body()

    nc.compile()
    return nc


def build_nc9(
    imgs,
    h,
    w,
    nw=512,
    s=124,
    hwloop=1,
    gT=4,
    xgbufs=3,
    xsbufs=2,
    zgbufs=3,
    zsbufs=2,
    pzbufs=8,
    dxouter=False,
):
    """build_nc8 with DMA batching: interior blocks' x loads are merged into
    one strided DMA per gT blocks (overlapping 4-row halos re-read from HBM)
    and z stores are merged per gT blocks, with DMA instructions alternating
    between the SP and ACT HW DGE rings. Measured per-instruction DMA cost
    (esp. the ~4us SBUF->HBM store completion) makes many small DMAs the
    bottleneck; merging + dual-ring cuts the DMA pass time under the PE time.
    """
    import concourse.bass as bass

    assert w % nw == 0 and nw <= 512
    nb = -(-h // s)
    m2 = s
    ms = 128
    nch = w // nw
    xw = w + 4
    r0_last = (nb - 1) * s
    rows_last = h - (r0_last - 2)
    k_last = rows_last + 2

    nc = bacc.Bacc("TRN2", target_bir_lowering=False, debug=False)
    x_d = nc.dram_tensor("x", [imgs, h, w], BF16, kind="ExternalInput")
    c_d = nc.dram_tensor("b1", [128, 15 * ms], BF16, kind="ExternalInput")
    d_d = nc.dram_tensor("b2", [128, 6 * ms], BF16, kind="ExternalInput")
    z_d = nc.dram_tensor("z", [imgs, h, w], BF16, kind="ExternalOutput")

    # load groups: block 0 and nb-1 single (need memzeroed halo rows); the
    # interior 1..nb-2 in chunks of gT. store groups: 0..nb-2 in chunks of
    # gT (uniform 124-row blocks), nb-1 single (ragged 64-row tail).
    ld_group = {}  # b -> (b0, T) for interior merged loads
    for b0 in range(1, nb - 1, gT):
        T = min(gT, nb - 1 - b0)
        for b in range(b0, b0 + T):
            ld_group[b] = (b0, T)
    st_group = {}  # b -> (b0, T)
    for b0 in range(0, nb - 1, gT):
        T = min(gT, nb - 1 - b0)
        for b in range(b0, b0 + T):
            st_group[b] = (b0, T)

    with tile.TileContext(nc) as tc:
        with (
            tc.tile_pool(name="const", bufs=1) as cpool,
            tc.tile_pool(name="xg", bufs=xgbufs) as xgpool,
            tc.tile_pool(name="xs", bufs=xsbufs) as xspool,
            tc.tile_pool(name="zg", bufs=zgbufs) as zgpool,
            tc.tile_pool(name="zs", bufs=zsbufs) as zspool,
            tc.tile_pool(name="pzp", bufs=pzbufs, space="PSUM") as pzpool,
        ):
            c_t = cpool.tile([128, 15 * ms], BF16)
            d_t = cpool.tile([128, 6 * ms], BF16)
            nc.sync.dma_start(out=c_t[:], in_=c_d[:])
            nc.sync.dma_start(out=d_t[:], in_=d_d[:])

            rings = [nc.sync, nc.scalar]
            ring_cnt = [0]

            def ring():
                e = rings[ring_cnt[0] & 1]
                ring_cnt[0] += 1
                return e

            def body():
                for g in range(imgs):
                    cur_x = None  # (b0, tile)
                    cur_z = None  # (b0, T, tile)
                    for b in range(nb):
                        r0 = b * s
                        # ---- x load ----
                        if b in ld_group:
                            b0, T = ld_group[b]
                            if b == b0:
                                xg = xgpool.tile([128, T, xw], BF16, tag="xg")
                                lo0 = b0 * s - 2
                                base = x_d[0, 0:1, 0:1]
                                src = bass.AP(
                                    tensor=base.tensor,
                                    offset=(g * h + lo0) * w,
                                    ap=[[w, 128], [s * w, T], [1, w]],
                                )
                                ring().dma_start(out=xg[:, :, 2 : 2 + w], in_=src)
                                nc.vector.memzero(xg[:, :, 0:2])
                                nc.vector.memzero(xg[:, :, 2 + w : 4 + w])
                                cur_x = (b0, xg)
                            b0x, xg = cur_x
                            x_t = xg[:, b - b0x, :]
                            k = 128
                        else:
                            lo, hi = max(r0 - 2, 0), min(r0 + s + 2, h)
                            p0, rows = lo - (r0 - 2), hi - lo
                            x_t = xspool.tile([128, xw], BF16, tag="xs")
                            if b == nb - 1:
                                zp0 = max(p for p in (0, 32, 64, 96) if p <= rows)
                                for q0, q1 in ((0, 32), (32, 64), (64, 128)):
                                    if q1 > zp0:
                                        nc.vector.memzero(x_t[max(q0, zp0) : q1, :])
                            ring().dma_start(
                                out=x_t[p0 : p0 + rows, 2 : 2 + w], in_=x_d[g, lo:hi, :]
                            )
                            if b == 0:
                                nc.vector.memzero(x_t[0:2, :])
                            nc.vector.memzero(x_t[:, 0:2])
                            nc.vector.memzero(x_t[:, 2 + w : 4 + w])
                            k = k_last if b == nb - 1 else 128
                        v = 0 if b == 0 else (2 if b == nb - 1 else 1)

                        # ---- z tile ----
                        if b in st_group:
                            b0z, Tz = st_group[b]
                            if b == b0z:
                                cur_z = (b0z, Tz, zgpool.tile([m2, Tz, w], BF16, tag="zg", name="zg"))
                            _, _, zg = cur_z
                            z_view = zg[:, b - b0z, :]
                        else:
                            z_view = zspool.tile([m2, w], BF16, tag="zs")

                        # ---- matmuls + copies ----
                        def copy_out(j, pz):
                            if j % 2 == 0:
                                nc.scalar.copy(
                                    out=z_view[:, nw * j : nw * j + nw], in_=pz[0:m2, :]
                                )
                            else:
                                nc.vector.tensor_copy(
                                    out=z_view[:, nw * j : nw * j + nw], in_=pz[0:m2, :]
                                )

                        if not dxouter:
                            for j in range(nch):
                                pz = pzpool.tile([ms, nw], F32, tag="pz")
                                corrj = (j == 0) or (j == nch - 1)
                                for dx in range(5):
                                    nc.tensor.matmul(
                                        pz[:],
                                        c_t[0:k, (v * 5 + dx) * ms : (v * 5 + dx + 1) * ms],
                                        x_t[0:k, nw * j + dx : nw * j + dx + nw],
                                        start=(dx == 0),
                                        stop=(dx == 4 and not corrj),
                                    )
                                if corrj and j == 0:
                                    nc.tensor.matmul(
                                        pz[:, 0:2],
                                        d_t[0:k, (v * 2 + 0) * ms : (v * 2 + 1) * ms],
                                        x_t[0:k, 2:0:-1],
                                        start=False,
                                        stop=(j != nch - 1),
                                    )
                                if corrj and j == nch - 1:
                                    nc.tensor.matmul(
                                        pz[:, nw - 2 : nw],
                                        d_t[0:k, (v * 2 + 1) * ms : (v * 2 + 2) * ms],
                                        x_t[0:k, w + 2 : w : -1],
                                        start=False,
                                        stop=True,
                                    )
                                copy_out(j, pz)
                        else:
                            pzs = [pzpool.tile([ms, nw], F32, tag="pz", name=f"pz{j}") for j in range(nch)]
                            for dx in range(5):
                                for j in range(nch):
                                    nc.tensor.matmul(
                                        pzs[j][:],
                                        c_t[0:k, (v * 5 + dx) * ms : (v * 5 + dx + 1) * ms],
                                        x_t[0:k, nw * j + dx : nw * j + dx + nw],
                                        start=(dx == 0),
                                        stop=(dx == 4 and j not in (0, nch - 1)),
                                    )
                            nc.tensor.matmul(
                                pzs[0][:, 0:2],
                                d_t[0:k, (v * 2 + 0) * ms : (v * 2 + 1) * ms],
                                x_t[0:k, 2:0:-1],
                                start=False,
                                stop=True,
                            )
                            nc.tensor.matmul(
                                pzs[nch - 1][:, nw - 2 : nw],
                                d_t[0:k, (v * 2 + 1) * ms : (v * 2 + 2) * ms],
                                x_t[0:k, w + 2 : w : -1],
                                start=False,
                                stop=True,
                            )
                            for j in range(nch):
                                copy_out(j, pzs[j])

                        # ---- z store ----
                        if b in st_group:
                            b0z, Tz, zg = cur_z
                            if b == b0z + Tz - 1:
                                zbase = z_d[0, 0:1, 0:1]
                                dst = bass.AP(
                                    tensor=zbase.tensor,
                                    offset=(g * h + b0z * s) * w,
                                    ap=[[w, m2], [s * w, Tz], [1, w]],
                                )
                                ring().dma_start(out=dst, in_=zg[:, :, :])
                        else:
                            rows_out = min(s, h - r0)
                            ring().dma_start(
                                out=z_d[g, r0 : r0 + rows_out, :],
                                in_=z_view[0:rows_out, :],
                            )

            if hwloop > 1:
                with tc.For_i(0, hwloop):
                    body()
            else:
                body()

    nc.compile()
    return nc


def build_nc10(imgs, h, w, nw=512, s=124, hwloop=1, xbufs=8, zbufs=6, pzbufs=8):
    """build_nc8 (per-block DMAs, good DRAM locality) with each block's x
    load and z store alternating between the SP and ACT HW DGE rings, so
    the ~4us per-store completion cost is paid on two rings in parallel."""
    assert w % nw == 0 and nw <= 512
    nb = -(-h // s)
    m2 = s
    ms = 128
    nch = w // nw
    xw = w + 4
    r0_last = (nb - 1) * s
    rows_last = h - (r0_last - 2)
    k_last = rows_last + 2

    nc = bacc.Bacc("TRN2", target_bir_lowering=False, debug=False)
    x_d = nc.dram_tensor("x", [imgs, h, w], BF16, kind="ExternalInput")
    c_d = nc.dram_tensor("b1", [128, 15 * ms], BF16, kind="ExternalInput")
    d_d = nc.dram_tensor("b2", [128, 6 * ms], BF16, kind="ExternalInput")
    z_d = nc.dram_tensor("z", [imgs, h, w], BF16, kind="ExternalOutput")

    blocks = [(g, b) for g in range(imgs) for b in range(nb)]

    with tile.TileContext(nc) as tc:
        with (
            tc.tile_pool(name="const", bufs=1) as cpool,
            tc.tile_pool(name="xp", bufs=xbufs) as xpool,
            tc.tile_pool(name="zp", bufs=zbufs) as zpool,
            tc.tile_pool(name="pzp", bufs=pzbufs, space="PSUM") as pzpool,
        ):
            c_t = cpool.tile([128, 15 * ms], BF16)
            d_t = cpool.tile([128, 6 * ms], BF16)
            nc.sync.dma_start(out=c_t[:], in_=c_d[:])
            nc.sync.dma_start(out=d_t[:], in_=d_d[:])

            def body():
                for t, (g, b) in enumerate(blocks):
                    r0 = b * s
                    lo, hi = max(r0 - 2, 0), min(r0 + s + 2, h)
                    p0, rows = lo - (r0 - 2), hi - lo
                    x_t = xpool.tile([128, xw], BF16, tag="x")
                    if b == nb - 1:
                        zp0 = max(p for p in (0, 32, 64, 96) if p <= rows)
                        for q0, q1 in ((0, 32), (32, 64), (64, 128)):
                            if q1 > zp0:
                                nc.vector.memzero(x_t[max(q0, zp0) : q1, :])
                    ld_eng = nc.sync if t % 2 == 0 else nc.scalar
                    st_eng = nc.scalar if t % 2 == 0 else nc.sync
                    ld_eng.dma_start(
                        out=x_t[p0 : p0 + rows, 2 : 2 + w], in_=x_d[g, lo:hi, :]
                    )
                    if b == 0:
                        nc.vector.memzero(x_t[0:2, :])
                    nc.vector.memzero(x_t[:, 0:2])
                    nc.vector.memzero(x_t[:, 2 + w : 4 + w])
                    k = k_last if b == nb - 1 else 128
                    v = 0 if b == 0 else (2 if b == nb - 1 else 1)
                    rows_out = min(s, h - r0)
                    z_t = zpool.tile([m2, w], BF16, tag="z")
                    for j in range(nch):
                        pz = pzpool.tile([ms, nw], F32, tag="pz")
                        corrj = (j == 0) or (j == nch - 1)
                        for dx in range(5):
                            nc.tensor.matmul(
                                pz[:],
                                c_t[0:k, (v * 5 + dx) * ms : (v * 5 + dx + 1) * ms],
                                x_t[0:k, nw * j + dx : nw * j + dx + nw],
                                start=(dx == 0),
                                stop=(dx == 4 and not corrj),
                            )
                        if corrj and j == 0:
                            nc.tensor.matmul(
                                pz[:, 0:2],
                                d_t[0:k, (v * 2 + 0) * ms : (v * 2 + 1) * ms],
                                x_t[0:k, 2:0:-1],
                                start=False,
                                stop=(j != nch - 1),
                            )
                        if corrj and j == nch - 1:
                            nc.tensor.matmul(
                                pz[:, nw - 2 : nw],
                                d_t[0:k, (v * 2 + 1) * ms : (v * 2 + 2) * ms],
                                x_t[0:k, w + 2 : w : -1],
                                start=False,
                                stop=True,
                            )
                        if j % 2 == 0:
                            nc.scalar.copy(
                                out=z_t[:, nw * j : nw * j + nw], in_=pz[0:m2, :]
                            )
                        else:
                            nc.vector.tensor_copy(
                                out=z_t[:, nw * j : nw * j + nw], in_=pz[0:m2, :]
                            )
                    st_eng.dma_start(
                        out=z_d[g, r0 : r0 + rows_out, :], in_=z_t[0:rows_out, :]
                    )

            if hwloop > 1:
                with tc.For_i(0, hwloop):
                    body()
            else:
                body()

    nc.compile()
    return nc


def build_nc11(
    imgs, h, w, nw=512, s=124, hwloop=1, xbufs=8, zbufs=6, pzbufs=8, gp_every=3
):
    """build_nc10 + every gp_every-th z store issued via the gpsimd SWDGE,
    adding a third independent DMA path (~37 GB/s) to the ~93 GB/s HWDGE
    store cap that paces the whole kernel."""
    assert w % nw == 0 and nw <= 512
    nb = -(-h // s)
    m2 = s
    ms = 128
    nch = w // nw
    xw = w + 4
    r0_last = (nb - 1) * s
    rows_last = h - (r0_last - 2)
    k_last = rows_last + 2

    nc = bacc.Bacc("TRN2", target_bir_lowering=False, debug=False)
    x_d = nc.dram_tensor("x", [imgs, h, w], BF16, kind="ExternalInput")
    c_d = nc.dram_tensor("b1", [128, 15 * ms], BF16, kind="ExternalInput")
    d_d = nc.dram_tensor("b2", [128, 6 * ms], BF16, kind="ExternalInput")
    z_d = nc.dram_tensor("z", [imgs, h, w], BF16, kind="ExternalOutput")

    blocks = [(g, b) for g in range(imgs) for b in range(nb)]

    with tile.TileContext(nc) as tc:
        with (
            tc.tile_pool(name="const", bufs=1) as cpool,
            tc.tile_pool(name="xp", bufs=xbufs) as xpool,
            tc.tile_pool(name="zp", bufs=zbufs) as zpool,
            tc.tile_pool(name="pzp", bufs=pzbufs, space="PSUM") as pzpool,
        ):
            c_t = cpool.tile([128, 15 * ms], BF16)
            d_t = cpool.tile([128, 6 * ms], BF16)
            nc.sync.dma_start(out=c_t[:], in_=c_d[:])
            nc.sync.dma_start(out=d_t[:], in_=d_d[:])

            def body():
                for t, (g, b) in enumerate(blocks):
                    r0 = b * s
                    lo, hi = max(r0 - 2, 0), min(r0 + s + 2, h)
                    p0, rows = lo - (r0 - 2), hi - lo
                    x_t = xpool.tile([128, xw], BF16, tag="x")
                    if b == nb - 1:
                        zp0 = max(p for p in (0, 32, 64, 96) if p <= rows)
                        for q0, q1 in ((0, 32), (32, 64), (64, 128)):
                            if q1 > zp0:
                                nc.vector.memzero(x_t[max(q0, zp0) : q1, :])
                    ld_eng = nc.sync if t % 2 == 0 else nc.scalar
                    ld_eng.dma_start(
                        out=x_t[p0 : p0 + rows, 2 : 2 + w], in_=x_d[g, lo:hi, :]
                    )
                    if b == 0:
                        nc.vector.memzero(x_t[0:2, :])
                    nc.vector.memzero(x_t[:, 0:2])
                    nc.vector.memzero(x_t[:, 2 + w : 4 + w])
                    k = k_last if b == nb - 1 else 128
                    v = 0 if b == 0 else (2 if b == nb - 1 else 1)
                    rows_out = min(s, h - r0)
                    z_t = zpool.tile([m2, w], BF16, tag="z")
                    for j in range(nch):
                        pz = pzpool.tile([ms, nw], F32, tag="pz")
                        corrj = (j == 0) or (j == nch - 1)
                        for dx in range(5):
                            nc.tensor.matmul(
                                pz[:],
                                c_t[0:k, (v * 5 + dx) * ms : (v * 5 + dx + 1) * ms],
                                x_t[0:k, nw * j + dx : nw * j + dx + nw],
                                start=(dx == 0),
                                stop=(dx == 4 and not corrj),
                            )
                        if corrj and j == 0:
                            nc.tensor.matmul(
                                pz[:, 0:2],
                                d_t[0:k, (v * 2 + 0) * ms : (v * 2 + 1) * ms],
                                x_t[0:k, 2:0:-1],
                                start=False,
                                stop=(j != nch - 1),
                            )
                        if corrj and j == nch - 1:
                            nc.tensor.matmul(
                                pz[:, nw - 2 : nw],
                                d_t[0:k, (v * 2 + 1) * ms : (v * 2 + 2) * ms],
                                x_t[0:k, w + 2 : w : -1],
                                start=False,
                                stop=True,
                            )
                        if j % 2 == 0:
                            nc.scalar.copy(
                                out=z_t[:, nw * j : nw * j + nw], in_=pz[0:m2, :]
                            )
                        else:
                            nc.vector.tensor_copy(
                                out=z_t[:, nw * j : nw * j + nw], in_=pz[0:m2, :]
                            )
                    if gp_every and t % gp_every == gp_every - 1:
                        st_eng = nc.gpsimd
                    else:
                        st_eng = nc.scalar if t % 2 == 0 else nc.sync
                    st_eng.dma_start(
                        out=z_d[g, r0 : r0 + rows_out, :], in_=z_t[0:rows_out, :]
                    )

            if hwloop > 1:
                with tc.For_i(0, hwloop):
                    body()
            else:
                body()

    nc.compile()
    return nc


def _shift_bands_v0(b1, b2, m2):
    """Shift the top-block (variant 0) band sections up 2 k-rows so block 0
    can load x rows 0..126 at partition 0 with no zero-row padding: the
    k=0,1 taps (x rows -2,-1) multiplied zeros before; after the shift they
    simply don't exist."""
    c = b1.reshape(128, 3, 5, m2).copy()
    d = b2.reshape(128, 3, 2, m2).copy()
    c[:-2, 0] = c[2:, 0]
    c[-2:, 0] = 0.0
    d[:-2, 0] = d[2:, 0]
    d[-2:, 0] = 0.0
    return (
        np.ascontiguousarray(c.reshape(128, 15 * m2)),
        np.ascontiguousarray(d.reshape(128, 6 * m2)),
    )


def build_nc12(
    imgs,
    h,
    w,
    nw=512,
    s=124,
    hwloop=1,
    xbufs=10,
    zbufs=8,
    pzbufs=8,
    gp_every=3,
    bodies_per_iter=1,
):
    """build_nc11 with every memset eliminated from the block loop:

    - bands from _shift_bands_v0: block 0 loads x rows 0..s+2 at partition
      0 (k=126), no zero-row padding;
    - bottom block uses k=rows actually loaded (66) — the virtual zero rows
      below the image are simply not contracted;
    - the 4 halo columns of each x pool buffer are zeroed ONCE before the
      loop (pool rotation is static), never touched by the data DMAs, and
      read as zeros forever after.

    This removes all per-block DVE memsets and their DMA->memset->matmul
    dependency chains, and reduces DVE port pressure (which stalls gpsimd
    SWDGE descriptor generation).
    """
    assert w % nw == 0 and nw <= 512
    nb = -(-h // s)
    m2 = s
    ms = 128
    nch = w // nw
    xw = w + 4

    nc = bacc.Bacc("TRN2", target_bir_lowering=False, debug=False)
    x_d = nc.dram_tensor("x", [imgs, h, w], BF16, kind="ExternalInput")
    c_d = nc.dram_tensor("b1", [128, 15 * ms], BF16, kind="ExternalInput")
    d_d = nc.dram_tensor("b2", [128, 6 * ms], BF16, kind="ExternalInput")
    z_d = nc.dram_tensor("z", [imgs, h, w], BF16, kind="ExternalOutput")

    blocks = [(g, b) for g in range(imgs) for b in range(nb)]

    with tile.TileContext(nc) as tc:
        with (
            tc.tile_pool(name="const", bufs=1) as cpool,
            tc.tile_pool(name="xp", bufs=xbufs) as xpool,
            tc.tile_pool(name="zp", bufs=zbufs) as zpool,
            tc.tile_pool(name="pzp", bufs=pzbufs, space="PSUM") as pzpool,
        ):
            c_t = cpool.tile([128, 15 * ms], BF16)
            d_t = cpool.tile([128, 6 * ms], BF16)
            nc.sync.dma_start(out=c_t[:], in_=c_d[:])
            nc.sync.dma_start(out=d_t[:], in_=d_d[:])

            def body():
                for t, (g, b) in enumerate(blocks):
                    r0 = b * s
                    lo, hi = max(r0 - 2, 0), min(r0 + s + 2, h)
                    rows = hi - lo
                    k = rows
                    x_t = xpool.tile([128, xw], BF16, tag="x")
                    ld_eng = nc.sync if t % 2 == 0 else nc.scalar
                    ld_eng.dma_start(out=x_t[0:rows, 2 : 2 + w], in_=x_d[g, lo:hi, :])
                    nc.vector.memzero(x_t[:, 0:2])
                    nc.vector.memzero(x_t[:, 2 + w : 4 + w])
                    v = 0 if b == 0 else (2 if b == nb - 1 else 1)
                    rows_out = min(s, h - r0)
                    z_t = zpool.tile([m2, w], BF16, tag="z")
                    for j in range(nch):
                        pz = pzpool.tile([ms, nw], F32, tag="pz")
                        corrj = (j == 0) or (j == nch - 1)
                        for dx in range(5):
                            nc.tensor.matmul(
                                pz[:],
                                c_t[0:k, (v * 5 + dx) * ms : (v * 5 + dx + 1) * ms],
                                x_t[0:k, nw * j + dx : nw * j + dx + nw],
                                start=(dx == 0),
                                stop=(dx == 4 and not corrj),
                            )
                        if corrj and j == 0:
                            nc.tensor.matmul(
                                pz[:, 0:2],
                                d_t[0:k, (v * 2 + 0) * ms : (v * 2 + 1) * ms],
                                x_t[0:k, 2:0:-1],
                                start=False,
                                stop=(j != nch - 1),
                            )
                        if corrj and j == nch - 1:
                            nc.tensor.matmul(
                                pz[:, nw - 2 : nw],
                                d_t[0:k, (v * 2 + 1) * ms : (v * 2 + 2) * ms],
                                x_t[0:k, w + 2 : w : -1],
                                start=False,
                                stop=True,
                            )
                        if j % 2 == 0:
                            nc.scalar.copy(
                                out=z_t[:, nw * j : nw * j + nw], in_=pz[0:m2, :]
                            )
                        else:
                            nc.vector.tensor_copy(
                                out=z_t[:, nw * j : nw * j + nw], in_=pz[0:m2, :]
                            )
                    if gp_every and t % gp_every == gp_every - 1:
                        st_eng = nc.gpsimd
                    else:
                        st_eng = nc.scalar if t % 2 == 0 else nc.sync
                    st_eng.dma_start(
                        out=z_d[g, r0 : r0 + rows_out, :], in_=z_t[0:rows_out, :]
                    )

            if hwloop > 1:
                with tc.For_i(0, hwloop):
                    for _ in range(bodies_per_iter):
                        body()
            else:
                body()

    nc.compile()
    return nc


def _pad_bands_128(b1, b2, m2):
    c = b1.reshape(128, 15, m2)
    cp = np.zeros((128, 15, 128), b1.dtype)
    cp[:, :, :m2] = c
    d = b2.reshape(128, 6, m2)
    dp = np.zeros((128, 6, 128), b2.dtype)
    dp[:, :, :m2] = d
    return (
        np.ascontiguousarray(cp.reshape(128, 15 * 128)),
        np.ascontiguousarray(dp.reshape(128, 6 * 128)),
    )


def to_bf16(a):
    import ml_dtypes

    return np.ascontiguousarray(np.asarray(a).astype(ml_dtypes.bfloat16))


def make_in_maps(x, w1, w2, h=FULL_H, s=124):
    """bf16 per-core input maps (v0-shifted + 128-padded bands) from full
    f32 inputs; matches build_nc12."""
    nb = -(-h // s)
    b1, b2 = _build_bands5(w1, w2, h, s, nb)
    b1, b2 = _shift_bands_v0(b1, b2, s)
    b1, b2 = _pad_bands_128(b1, b2, s)
    b1, b2 = to_bf16(b1), to_bf16(b2)
    xb = to_bf16(np.asarray(x, np.float32).reshape(FULL_B, FULL_H, FULL_W))
    imgs = FULL_B // NCORES
    return [
        {"x": np.ascontiguousarray(xb[imgs * c : imgs * (c + 1)]), "b1": b1, "b2": b2}
        for c in range(NCORES)
    ]


_NC_CACHE = None


def kernel(x, w1, w2):
    global _NC_CACHE, LAST_RESULTS
    in_maps = make_in_maps(x, w1, w2)
    if _NC_CACHE is None:
        _NC_CACHE = build_nc12(FULL_B // NCORES, FULL_H, FULL_W, nw=512, s=124)
    nc = _NC_CACHE
    res = run_bass_kernel_spmd(nc, in_maps, core_ids=list(range(NCORES)), trace=TRACE)
    LAST_RESULTS = res
    out = np.stack(
        [np.asarray(res.results[c]["z"], np.float32) for c in range(NCORES)], axis=0
    )
    return out.reshape(FULL_B, 1, FULL_H, FULL_W)



# revision 21
# speedup vs baseline: 1.2749x; 1.0027x over previous
"""Trainium2 Bass kernel: two chained SAME-padded 3x3 single-channel convs.

  reference: z = conv3x3(conv3x3(x, w1), w2)   x: [16,1,2048,2048] f32

Shipped strategy (build_nc12; kernel() below): pure data parallel, 2
images per core on 8 cores, single fused pass per image band:
  - The two chained convs are folded on the host into ONE composite 5x5
    operator (exact, including the chained-SAME-padding semantics; see
    _build_bands5). Per 124-row output band the composite is 5
    accumulating TensorEngine matmuls (one per horizontal tap dx, the 5
    vertical taps encoded as banded stationary matrices), plus 2 tiny
    N=2 matmuls correcting the phantom intermediate column at the left/
    right image edges. The intermediate conv1 result never exists.
  - bf16 end-to-end (x, bands, z) with f32 PSUM accumulation: halves all
    HBM traffic; measured 4.0e-3 absmax-relative error vs the 2e-2 gate.
  - x loads and z stores alternate between the SP and ACT HW DGE rings,
    and every 3rd z store goes via gpsimd SWDGE: SBUF->HBM stores cap at
    ~93 GB/s on HWDGE total (measured; loads do ~252 GB/s), so the extra
    ~37 GB/s SWDGE path relieves the store wall that paces the kernel.
  - No memsets in the block loop: the top band uses k-shifted bands
    (_shift_bands_v0) instead of zeroed halo rows, the bottom band
    contracts only the k rows actually loaded, and only the 4 halo
    columns are zeroed per tile (2 small DVE memsets).
  - Optional hwloop (tc.For_i) repeats the whole pass inside one NEFF
    for dispatch-free steady-state timing (test.py).

Band matrices are built on the host from w1/w2 (they are just 9 floats
each) and passed as extra inputs. Older build_nc* variants are kept for
reference; kernel() uses build_nc12.
"""

import hashlib
import os
import shutil

import numpy as np

import concourse.mybir as mybir
import concourse.tile as tile
from concourse import bacc, bass2jax
from concourse.bass_utils import run_bass_kernel_spmd


def _install_neff_disk_cache():
    """Cache compiled NEFFs on disk keyed by BIR content hash — the
    neuronxcc backend takes minutes for this kernel and has no cache of
    its own, so a fresh process would otherwise recompile every run."""
    if getattr(bass2jax, "_ant_neff_cache_installed", False):
        return
    orig = bass2jax.compile_bir_kernel

    def cached(bir_json, tmpdir, neff_name="file.neff"):
        try:
            cdir = os.path.expanduser("~/.cache/bass_neff")
            os.makedirs(cdir, exist_ok=True)
            key = hashlib.sha256(
                bir_json if isinstance(bir_json, bytes) else bir_json.encode()
            ).hexdigest()[:32]
            cpath = os.path.join(cdir, f"{key}.neff")
            if os.path.exists(cpath):
                outdir = os.path.join(tmpdir, "sg00")
                os.makedirs(outdir, exist_ok=True)
                dst = os.path.join(outdir, neff_name)
                shutil.copyfile(cpath, dst)
                return dst
            neff = orig(bir_json, tmpdir, neff_name)
            shutil.copyfile(neff, cpath + ".tmp")
            os.replace(cpath + ".tmp", cpath)
            return neff
        except Exception:
            return orig(bir_json, tmpdir, neff_name)

    bass2jax.compile_bir_kernel = cached
    bass2jax._ant_neff_cache_installed = True


_install_neff_disk_cache()

F32 = mybir.dt.float32
F32R = mybir.dt.float32r

MM_DT = F32R  # matmul operand dtype: F32R (fast) or F32 (exact)

NCORES = 8
FULL_B, FULL_H, FULL_W = 16, 2048, 2048

TRACE = False  # set True (from test harness) to capture an NTFF profile
LAST_RESULTS = None  # BassKernelResults of the most recent run


def _build_bands(w1, w2, h, s, nb):
    """Host-side band matrices for the vertical taps.

    B1: [128, 3 variants, 3 dx, 126]; variant 0 = top block, 1 = mid,
    2 = bottom block.  B2: [126, 3 dx, 124].
    """
    W1 = np.asarray(w1, np.float32).reshape(3, 3)
    W2 = np.asarray(w2, np.float32).reshape(3, 3)
    m1, m2 = s + 2, s
    b1 = np.zeros((128, 3, 3, m1), np.float32)
    b2 = np.zeros((m1, 3, m2), np.float32)
    mm = np.arange(m1)
    for i in range(3):
        for dx in range(3):
            b1[mm + i, :, dx, mm] = W1[i, dx]
    mm = np.arange(m2)
    for i in range(3):
        for dx in range(3):
            b2[mm + i, dx, mm] = W2[i, dx]
    # top block: y row r0-1 = -1 is conv2's zero padding, not a computed row
    b1[:, 0, :, 0] = 0.0
    # bottom block: y row == h is zero padding
    r0_last = (nb - 1) * s
    b1[:, 2, :, h - r0_last + 1] = 0.0
    return np.ascontiguousarray(b1.reshape(128, 9 * m1)), np.ascontiguousarray(
        b2.reshape(m1, 3 * m2)
    )


def build_nc(imgs, h, w, nw=512, s=124, repeat=1):
    """Build the per-core Bass program (parametric so a small config can
    be validated in CoreSim)."""
    assert w % nw == 0 and nw <= 512
    nb = -(-h // s)  # blocks per image
    m1, m2 = s + 2, s  # conv1/conv2 output rows per block
    nch = w // nw  # width chunks
    xw = w + 4  # tile width: col 0 zero | 1..w data | w+1 zero | pad
    r0_last = (nb - 1) * s
    rows_last = h - (r0_last - 2)  # x rows loaded for the last block
    k1_last = rows_last + 2

    nc = bacc.Bacc("TRN2", target_bir_lowering=False, debug=False)
    x_d = nc.dram_tensor("x", [imgs, h, w], MM_DT, kind="ExternalInput")
    b1_d = nc.dram_tensor("b1", [128, 9 * m1], MM_DT, kind="ExternalInput")
    b2_d = nc.dram_tensor("b2", [m1, 3 * m2], MM_DT, kind="ExternalInput")
    z_d = nc.dram_tensor("z", [imgs, h, w], F32, kind="ExternalOutput")

    blocks = [(g, b) for g in range(imgs) for b in range(nb)] * repeat

    with tile.TileContext(nc) as tc:
        with (
            tc.tile_pool(name="const", bufs=1) as cpool,
            tc.tile_pool(name="xp", bufs=3) as xpool,
            tc.tile_pool(name="yp", bufs=2) as ypool,
            tc.tile_pool(name="zp", bufs=2) as zpool,
            tc.tile_pool(name="pyp", bufs=4, space="PSUM") as pypool,
            tc.tile_pool(name="pzp", bufs=4, space="PSUM") as pzpool,
        ):
            b1_t = cpool.tile([128, 9 * m1], MM_DT)
            b2_t = cpool.tile([m1, 3 * m2], MM_DT)
            nc.sync.dma_start(out=b1_t[:], in_=b1_d[:])
            nc.sync.dma_start(out=b2_t[:], in_=b2_d[:])

            pend = None  # (img, block, y_tile) awaiting conv2
            for t in range(len(blocks) + 1):
                if t < len(blocks):
                    g, b = blocks[t]
                    r0 = b * s
                    lo, hi = max(r0 - 2, 0), min(r0 + m1, h)
                    p0, rows = lo - (r0 - 2), hi - lo
                    x_t = xpool.tile([128, xw], MM_DT, tag="x")
                    if b == nb - 1:
                        # bottom block: zero the whole tile first (covers the
                        # 2 halo rows below the image and the halo columns);
                        # compute-engine APs can only start at partition
                        # 0/32/64/96, so a targeted halo-row memset is not
                        # expressible.
                        nc.vector.memzero(x_t[:, :])
                    nc.sync.dma_start(
                        out=x_t[p0 : p0 + rows, 1 : 1 + w], in_=x_d[g, lo:hi, :]
                    )
                    if b == 0:
                        nc.vector.memzero(x_t[0:2, :])
                    if b != nb - 1:
                        nc.vector.memzero(x_t[:, 0:1])
                        nc.vector.memzero(x_t[:, 1 + w : 2 + w])
                    k1 = k1_last if b == nb - 1 else 128
                    v = 0 if b == 0 else (2 if b == nb - 1 else 1)
                    y_t = ypool.tile([m1, xw], MM_DT, tag="y")
                    for j in range(nch):
                        py = pypool.tile([m1, nw], F32, tag="py")
                        for dx in range(3):
                            nc.tensor.matmul(
                                py[:],
                                b1_t[0:k1, (v * 3 + dx) * m1 : (v * 3 + dx + 1) * m1],
                                x_t[0:k1, nw * j + dx : nw * j + dx + nw],
                                start=(dx == 0),
                                stop=(dx == 2),
                            )
                        nc.vector.tensor_copy(
                            out=y_t[:, 1 + nw * j : 1 + nw * j + nw], in_=py[:]
                        )
                    nc.vector.memzero(y_t[:, 0:1])
                    nc.vector.memzero(y_t[:, 1 + w : 2 + w])
                    pend_next = (g, b, y_t)
                else:
                    pend_next = None

                if pend is not None:
                    g2, b2i, y_prev = pend
                    r0 = b2i * s
                    rows = min(s, h - r0)
                    z_t = zpool.tile([m2, w], F32, tag="z")
                    for j in range(nch):
                        pz = pzpool.tile([m2, nw], F32, tag="pz")
                        for dx in range(3):
                            nc.tensor.matmul(
                                pz[:],
                                b2_t[0:m1, dx * m2 : (dx + 1) * m2],
                                y_prev[0:m1, nw * j + dx : nw * j + dx + nw],
                                start=(dx == 0),
                                stop=(dx == 2),
                            )
                        nc.scalar.copy(out=z_t[:, nw * j : nw * j + nw], in_=pz[:])
                    nc.sync.dma_start(out=z_d[g2, r0 : r0 + rows, :], in_=z_t[0:rows, :])
                pend = pend_next

    nc.compile()
    return nc


def _build_bands5(w1, w2, h, s, nb):
    """Composite single-pass operator: z = C(x) where C = conv2 o conv1
    with the chained-SAME-padding semantics folded in exactly.

    Vertical behavior (including the y[-1]/y[h] zero rows and the image
    top/bottom) is encoded in per-variant 5-diagonal band matrices
    C[k, v, dx, m].  The only horizontal discrepancy of the composite
    vs the chained convs is the phantom y column at each side; D holds
    the two exact correction bands (applied to x col 0 / w-1, adding
    into z col 0 / w-1).
    """
    W1 = np.asarray(w1, np.float64).reshape(3, 3)
    W2 = np.asarray(w2, np.float64).reshape(3, 3)
    m1, m2 = s + 2, s

    def a_mat(col, rows, cols):
        a = np.zeros((rows, cols), np.float64)
        r = np.arange(rows)
        for i in range(3):
            a[r, r + i] = col[i]
        return a

    r0_last = (nb - 1) * s
    c = np.zeros((128, 3, 5, m2), np.float64)
    d = np.zeros((128, 3, 2, m2), np.float64)
    for v in range(3):
        a1 = [a_mat(W1[:, j], m1, 128) for j in range(3)]
        if v == 0:
            for a in a1:
                a[0, :] = 0.0  # y row -1 is conv2 zero padding
        if v == 2:
            for a in a1:
                a[h - r0_last + 1, :] = 0.0  # y row h is zero padding
        a2 = [a_mat(W2[:, j], m2, m1) for j in range(3)]
        for j in range(3):
            for jp in range(3):
                c[:, v, j + jp, :] += (a2[jp] @ a1[j]).T
        d[:, v, 0, :] = -(a2[0] @ a1[2]).T
        d[:, v, 1, :] = -(a2[2] @ a1[0]).T
    return (
        np.ascontiguousarray(c.reshape(128, 15 * m2).astype(np.float32)),
        np.ascontiguousarray(d.reshape(128, 6 * m2).astype(np.float32)),
    )


def build_nc5(imgs, h, w, nw=512, s=124, repeat=1, xbufs=4, zbufs=3, pzbufs=8, zdma="sync", corr=True):
    """Single-pass composite-5x5 program (see _build_bands5)."""
    assert w % nw == 0 and nw <= 512
    nb = -(-h // s)
    m2 = s
    nch = w // nw
    xw = w + 4  # cols 0,1 zero | 2..w+1 data | w+2,w+3 zero
    r0_last = (nb - 1) * s
    rows_last = h - (r0_last - 2)
    k_last = rows_last + 2

    nc = bacc.Bacc("TRN2", target_bir_lowering=False, debug=False)
    x_d = nc.dram_tensor("x", [imgs, h, w], MM_DT, kind="ExternalInput")
    c_d = nc.dram_tensor("b1", [128, 15 * m2], MM_DT, kind="ExternalInput")
    d_d = nc.dram_tensor("b2", [128, 6 * m2], MM_DT, kind="ExternalInput")
    z_d = nc.dram_tensor("z", [imgs, h, w], F32, kind="ExternalOutput")

    blocks = [(g, b) for g in range(imgs) for b in range(nb)] * repeat

    with tile.TileContext(nc) as tc:
        with (
            tc.tile_pool(name="const", bufs=1) as cpool,
            tc.tile_pool(name="xp", bufs=xbufs) as xpool,
            tc.tile_pool(name="zp", bufs=zbufs) as zpool,
            tc.tile_pool(name="pzp", bufs=pzbufs, space="PSUM") as pzpool,
        ):
            c_t = cpool.tile([128, 15 * m2], MM_DT)
            d_t = cpool.tile([128, 6 * m2], MM_DT)
            nc.sync.dma_start(out=c_t[:], in_=c_d[:])
            nc.sync.dma_start(out=d_t[:], in_=d_d[:])

            for g, b in blocks:
                r0 = b * s
                lo, hi = max(r0 - 2, 0), min(r0 + s + 2, h)
                p0, rows = lo - (r0 - 2), hi - lo
                x_t = xpool.tile([128, xw], MM_DT, tag="x")
                if b == nb - 1:
                    nc.vector.memzero(x_t[:, :])
                nc.sync.dma_start(
                    out=x_t[p0 : p0 + rows, 2 : 2 + w], in_=x_d[g, lo:hi, :]
                )
                if b == 0:
                    nc.vector.memzero(x_t[0:2, :])
                if b != nb - 1:
                    nc.vector.memzero(x_t[:, 0:2])
                    nc.vector.memzero(x_t[:, 2 + w : 4 + w])
                k = k_last if b == nb - 1 else 128
                v = 0 if b == 0 else (2 if b == nb - 1 else 1)
                rows_out = min(s, h - r0)
                z_t = zpool.tile([m2, w], F32, tag="z")
                for j in range(nch):
                    pz = pzpool.tile([m2, nw], F32, tag="pz")
                    corrj = corr and ((j == 0) or (j == nch - 1))
                    for dx in range(5):
                        nc.tensor.matmul(
                            pz[:],
                            c_t[0:k, (v * 5 + dx) * m2 : (v * 5 + dx + 1) * m2],
                            x_t[0:k, nw * j + dx : nw * j + dx + nw],
                            start=(dx == 0),
                            stop=(dx == 4 and not corrj),
                        )
                    # fp32r matmuls need an even moving-operand count and an
                    # 8B-aligned even-count dst, so the 1-column corrections
                    # run as N=2 with the partner column reading a zeroed
                    # halo column of x (negative-step AP) -> contributes 0.
                    if corrj and j == 0:
                        nc.tensor.matmul(
                            pz[:, 0:2],
                            d_t[0:k, (v * 2 + 0) * m2 : (v * 2 + 1) * m2],
                            x_t[0:k, 2:0:-1],  # cols [x 0, zero]
                            start=False,
                            stop=(j != nch - 1),
                        )
                    if corrj and j == nch - 1:
                        nc.tensor.matmul(
                            pz[:, nw - 2 : nw],
                            d_t[0:k, (v * 2 + 1) * m2 : (v * 2 + 2) * m2],
                            x_t[0:k, w + 2 : w : -1],  # cols [zero, x w-1]
                            start=False,
                            stop=True,
                        )
                    if j % 2 == 0:
                        nc.scalar.copy(out=z_t[:, nw * j : nw * j + nw], in_=pz[:])
                    else:
                        nc.vector.tensor_copy(
                            out=z_t[:, nw * j : nw * j + nw], in_=pz[:]
                        )
                zeng = nc.scalar if zdma == "scalar" else nc.sync
                zeng.dma_start(
                    out=z_d[g, r0 : r0 + rows_out, :], in_=z_t[0:rows_out, :]
                )

    nc.compile()
    return nc


def build_nc6(
    imgs, h, w, nw=512, s=124, repeat=1, xbufs=4, zbufs=3, pzbufs=8, zdma="scalar"
):
    """Composite single-pass program over HOST-PADDED x.

    x arrives as [imgs, h+4, w+4] with 2 zero rows/cols on every side,
    so the device needs no halo memsets at all: each band of s output
    rows is one clean [<=128, w+4] DMA, 22 accumulating fp32r matmuls,
    4 PSUM->SBUF copies and one store."""
    assert w % nw == 0 and nw <= 512
    nb = -(-h // s)
    m2 = s
    nch = w // nw
    xw = w + 4
    r0_last = (nb - 1) * s
    k_last = h + 4 - r0_last  # padded rows available for the last block

    nc = bacc.Bacc("TRN2", target_bir_lowering=False, debug=False)
    x_d = nc.dram_tensor("x", [imgs, h + 4, w + 4], MM_DT, kind="ExternalInput")
    c_d = nc.dram_tensor("b1", [128, 15 * m2], MM_DT, kind="ExternalInput")
    d_d = nc.dram_tensor("b2", [128, 6 * m2], MM_DT, kind="ExternalInput")
    z_d = nc.dram_tensor("z", [imgs, h, w], F32, kind="ExternalOutput")

    blocks = [(g, b) for g in range(imgs) for b in range(nb)] * repeat

    with tile.TileContext(nc) as tc:
        with (
            tc.tile_pool(name="const", bufs=1) as cpool,
            tc.tile_pool(name="xp", bufs=xbufs) as xpool,
            tc.tile_pool(name="zp", bufs=zbufs) as zpool,
            tc.tile_pool(name="pzp", bufs=pzbufs, space="PSUM") as pzpool,
        ):
            c_t = cpool.tile([128, 15 * m2], MM_DT)
            d_t = cpool.tile([128, 6 * m2], MM_DT)
            nc.sync.dma_start(out=c_t[:], in_=c_d[:])
            nc.sync.dma_start(out=d_t[:], in_=d_d[:])

            for g, b in blocks:
                r0 = b * s
                k = k_last if b == nb - 1 else 128
                x_t = xpool.tile([128, xw], MM_DT, tag="x")
                nc.sync.dma_start(out=x_t[0:k, :], in_=x_d[g, r0 : r0 + k, :])
                v = 0 if b == 0 else (2 if b == nb - 1 else 1)
                rows_out = min(s, h - r0)
                z_t = zpool.tile([m2, w], F32, tag="z")
                for j in range(nch):
                    pz = pzpool.tile([m2, nw], F32, tag="pz")
                    corrj = (j == 0) or (j == nch - 1)
                    for dx in range(5):
                        nc.tensor.matmul(
                            pz[:],
                            c_t[0:k, (v * 5 + dx) * m2 : (v * 5 + dx + 1) * m2],
                            x_t[0:k, nw * j + dx : nw * j + dx + nw],
                            start=(dx == 0),
                            stop=(dx == 4 and not corrj),
                        )
                    if j == 0:
                        nc.tensor.matmul(
                            pz[:, 0:2],
                            d_t[0:k, (v * 2 + 0) * m2 : (v * 2 + 1) * m2],
                            x_t[0:k, 2:0:-1],  # cols [x 0, zero]
                            start=False,
                            stop=(j != nch - 1),
                        )
                    if j == nch - 1:
                        nc.tensor.matmul(
                            pz[:, nw - 2 : nw],
                            d_t[0:k, (v * 2 + 1) * m2 : (v * 2 + 2) * m2],
                            x_t[0:k, w + 2 : w : -1],  # cols [zero, x w-1]
                            start=False,
                            stop=True,
                        )
                    if j % 2 == 0:
                        nc.scalar.copy(out=z_t[:, nw * j : nw * j + nw], in_=pz[:])
                    else:
                        nc.vector.tensor_copy(
                            out=z_t[:, nw * j : nw * j + nw], in_=pz[:]
                        )
                zeng = nc.scalar if zdma == "scalar" else nc.sync
                zeng.dma_start(
                    out=z_d[g, r0 : r0 + rows_out, :], in_=z_t[0:rows_out, :]
                )

    nc.compile()
    return nc


def pad_x(x, imgs, h, w):
    xp = np.zeros((imgs, h + 4, w + 4), np.float32)
    xp[:, 2 : h + 2, 2 : w + 2] = x
    return xp


BF16 = mybir.dt.bfloat16


def build_nc7(imgs, h, w, nw=512, s=124, hwloop=1, xbufs=6, zbufs=4, pzbufs=8):
    """bf16 single-pass composite-5x5 program (see _build_bands5).

    vs build_nc5: x/z/bands in bf16 (halves HBM traffic; PSUM accumulation
    stays f32), z stores issued from the Activation engine so loads (SP
    ring) and stores (ACT ring) use the two independent HW DGE queues, and
    an optional hardware loop (`hwloop` passes per NEFF execution) for
    dispatch-overhead-free steady-state timing.
    """
    assert w % nw == 0 and nw <= 512
    nb = -(-h // s)
    m2 = s
    nch = w // nw
    xw = w + 4  # cols 0,1 zero | 2..w+1 data | w+2,w+3 zero
    r0_last = (nb - 1) * s
    rows_last = h - (r0_last - 2)
    k_last = rows_last + 2

    nc = bacc.Bacc("TRN2", target_bir_lowering=False, debug=False)
    x_d = nc.dram_tensor("x", [imgs, h, w], BF16, kind="ExternalInput")
    c_d = nc.dram_tensor("b1", [128, 15 * m2], BF16, kind="ExternalInput")
    d_d = nc.dram_tensor("b2", [128, 6 * m2], BF16, kind="ExternalInput")
    z_d = nc.dram_tensor("z", [imgs, h, w], BF16, kind="ExternalOutput")

    blocks = [(g, b) for g in range(imgs) for b in range(nb)]

    with tile.TileContext(nc) as tc:
        with (
            tc.tile_pool(name="const", bufs=1) as cpool,
            tc.tile_pool(name="xp", bufs=xbufs) as xpool,
            tc.tile_pool(name="zp", bufs=zbufs) as zpool,
            tc.tile_pool(name="pzp", bufs=pzbufs, space="PSUM") as pzpool,
        ):
            c_t = cpool.tile([128, 15 * m2], BF16)
            d_t = cpool.tile([128, 6 * m2], BF16)
            nc.sync.dma_start(out=c_t[:], in_=c_d[:])
            nc.sync.dma_start(out=d_t[:], in_=d_d[:])

            def body():
                for g, b in blocks:
                    r0 = b * s
                    lo, hi = max(r0 - 2, 0), min(r0 + s + 2, h)
                    p0, rows = lo - (r0 - 2), hi - lo
                    x_t = xpool.tile([128, xw], BF16, tag="x")
                    if b == nb - 1:
                        nc.vector.memzero(x_t[:, :])
                    nc.sync.dma_start(
                        out=x_t[p0 : p0 + rows, 2 : 2 + w], in_=x_d[g, lo:hi, :]
                    )
                    if b == 0:
                        nc.vector.memzero(x_t[0:2, :])
                    if b != nb - 1:
                        nc.vector.memzero(x_t[:, 0:2])
                        nc.vector.memzero(x_t[:, 2 + w : 4 + w])
                    k = k_last if b == nb - 1 else 128
                    v = 0 if b == 0 else (2 if b == nb - 1 else 1)
                    rows_out = min(s, h - r0)
                    z_t = zpool.tile([m2, w], BF16, tag="z")
                    for j in range(nch):
                        pz = pzpool.tile([m2, nw], F32, tag="pz")
                        corrj = (j == 0) or (j == nch - 1)
                        for dx in range(5):
                            nc.tensor.matmul(
                                pz[:],
                                c_t[0:k, (v * 5 + dx) * m2 : (v * 5 + dx + 1) * m2],
                                x_t[0:k, nw * j + dx : nw * j + dx + nw],
                                start=(dx == 0),
                                stop=(dx == 4 and not corrj),
                            )
                        if corrj and j == 0:
                            nc.tensor.matmul(
                                pz[:, 0:2],
                                d_t[0:k, (v * 2 + 0) * m2 : (v * 2 + 1) * m2],
                                x_t[0:k, 2:0:-1],  # cols [x 0, zero]
                                start=False,
                                stop=(j != nch - 1),
                            )
                        if corrj and j == nch - 1:
                            nc.tensor.matmul(
                                pz[:, nw - 2 : nw],
                                d_t[0:k, (v * 2 + 1) * m2 : (v * 2 + 2) * m2],
                                x_t[0:k, w + 2 : w : -1],  # cols [zero, x w-1]
                                start=False,
                                stop=True,
                            )
                        if j % 2 == 0:
                            nc.scalar.copy(out=z_t[:, nw * j : nw * j + nw], in_=pz[:])
                        else:
                            nc.vector.tensor_copy(
                                out=z_t[:, nw * j : nw * j + nw], in_=pz[:]
                            )
                    nc.scalar.dma_start(
                        out=z_d[g, r0 : r0 + rows_out, :], in_=z_t[0:rows_out, :]
                    )

            if hwloop > 1:
                with tc.For_i(0, hwloop):
                    body()
            else:
                body()

    nc.compile()
    return nc


def build_nc8(imgs, h, w, nw=512, s=124, hwloop=1, xbufs=6, zbufs=4, pzbufs=8):
    """build_nc7 with the stationary band sections zero-padded from m2=124
    to 128 columns. A 128-column weight load triggers the PE's automatic
    Fast Weight Load path (2x for bf16); PSUM tiles grow to 128 partitions
    (rows 124..127 compute zeros) but still fit one 2KB bank."""
    assert w % nw == 0 and nw <= 512
    nb = -(-h // s)
    m2 = s
    ms = 128  # padded stationary columns / PSUM partitions
    nch = w // nw
    xw = w + 4
    r0_last = (nb - 1) * s
    rows_last = h - (r0_last - 2)
    k_last = rows_last + 2

    nc = bacc.Bacc("TRN2", target_bir_lowering=False, debug=False)
    x_d = nc.dram_tensor("x", [imgs, h, w], BF16, kind="ExternalInput")
    c_d = nc.dram_tensor("b1", [128, 15 * ms], BF16, kind="ExternalInput")
    d_d = nc.dram_tensor("b2", [128, 6 * ms], BF16, kind="ExternalInput")
    z_d = nc.dram_tensor("z", [imgs, h, w], BF16, kind="ExternalOutput")

    blocks = [(g, b) for g in range(imgs) for b in range(nb)]

    with tile.TileContext(nc) as tc:
        with (
            tc.tile_pool(name="const", bufs=1) as cpool,
            tc.tile_pool(name="xp", bufs=xbufs) as xpool,
            tc.tile_pool(name="zp", bufs=zbufs) as zpool,
            tc.tile_pool(name="pzp", bufs=pzbufs, space="PSUM") as pzpool,
        ):
            c_t = cpool.tile([128, 15 * ms], BF16)
            d_t = cpool.tile([128, 6 * ms], BF16)
            nc.sync.dma_start(out=c_t[:], in_=c_d[:])
            nc.sync.dma_start(out=d_t[:], in_=d_d[:])

            def body():
                for g, b in blocks:
                    r0 = b * s
                    lo, hi = max(r0 - 2, 0), min(r0 + s + 2, h)
                    p0, rows = lo - (r0 - 2), hi - lo
                    x_t = xpool.tile([128, xw], BF16, tag="x")
                    if b == nb - 1:
                        # only partitions >= rows hold stale data; compute-engine
                        # APs start at partition 0/32/64/96 and must not span
                        # more partitions than their alignment allows
                        zp0 = max(p for p in (0, 32, 64, 96) if p <= rows)
                        for q0, q1 in ((0, 32), (32, 64), (64, 128)):
                            if q1 > zp0:
                                nc.vector.memzero(x_t[max(q0, zp0) : q1, :])
                    nc.sync.dma_start(
                        out=x_t[p0 : p0 + rows, 2 : 2 + w], in_=x_d[g, lo:hi, :]
                    )
                    if b == 0:
                        nc.vector.memzero(x_t[0:2, :])
                    nc.vector.memzero(x_t[:, 0:2])
                    nc.vector.memzero(x_t[:, 2 + w : 4 + w])
                    k = k_last if b == nb - 1 else 128
                    v = 0 if b == 0 else (2 if b == nb - 1 else 1)
                    rows_out = min(s, h - r0)
                    z_t = zpool.tile([m2, w], BF16, tag="z")
                    for j in range(nch):
                        pz = pzpool.tile([ms, nw], F32, tag="pz")
                        corrj = (j == 0) or (j == nch - 1)
                        for dx in range(5):
                            nc.tensor.matmul(
                                pz[:],
                                c_t[0:k, (v * 5 + dx) * ms : (v * 5 + dx + 1) * ms],
                                x_t[0:k, nw * j + dx : nw * j + dx + nw],
                                start=(dx == 0),
                                stop=(dx == 4 and not corrj),
                            )
                        if corrj and j == 0:
                            nc.tensor.matmul(
                                pz[:, 0:2],
                                d_t[0:k, (v * 2 + 0) * ms : (v * 2 + 1) * ms],
                                x_t[0:k, 2:0:-1],  # cols [x 0, zero]
                                start=False,
                                stop=(j != nch - 1),
                            )
                        if corrj and j == nch - 1:
                            nc.tensor.matmul(
                                pz[:, nw - 2 : nw],
                                d_t[0:k, (v * 2 + 1) * ms : (v * 2 + 2) * ms],
                                x_t[0:k, w + 2 : w : -1],  # cols [zero, x w-1]
                                start=False,
                                stop=True,
                            )
                        if j % 2 == 0:
                            nc.scalar.copy(
                                out=z_t[:, nw * j : nw * j + nw], in_=pz[0:m2, :]
                            )
                        else:
                            nc.vector.tensor_copy(
                                out=z_t[:, nw * j : nw * j + nw], in_=pz[0:m2, :]
                            )
                    nc.scalar.dma_start(
                        out=z_d[g, r0 : r0 + rows_out, :], in_=z_t[0:rows_out, :]
                    )

            if hwloop > 1:
                with tc.For_i(0, hwloop):
                    body()
            else:
                body()

    nc.compile()
    return nc


def build_nc9(
    imgs,
    h,
    w,
    nw=512,
    s=124,
    hwloop=1,
    gT=4,
    xgbufs=3,
    xsbufs=2,
    zgbufs=3,
    zsbufs=2,
    pzbufs=8,
    dxouter=False,
):
    """build_nc8 with DMA batching: interior blocks' x loads are merged into
    one strided DMA per gT blocks (overlapping 4-row halos re-read from HBM)
    and z stores are merged per gT blocks, with DMA instructions alternating
    between the SP and ACT HW DGE rings. Measured per-instruction DMA cost
    (esp. the ~4us SBUF->HBM store completion) makes many small DMAs the
    bottleneck; merging + dual-ring cuts the DMA pass time under the PE time.
    """
    import concourse.bass as bass

    assert w % nw == 0 and nw <= 512
    nb = -(-h // s)
    m2 = s
    ms = 128
    nch = w // nw
    xw = w + 4
    r0_last = (nb - 1) * s
    rows_last = h - (r0_last - 2)
    k_last = rows_last + 2

    nc = bacc.Bacc("TRN2", target_bir_lowering=False, debug=False)
    x_d = nc.dram_tensor("x", [imgs, h, w], BF16, kind="ExternalInput")
    c_d = nc.dram_tensor("b1", [128, 15 * ms], BF16, kind="ExternalInput")
    d_d = nc.dram_tensor("b2", [128, 6 * ms], BF16, kind="ExternalInput")
    z_d = nc.dram_tensor("z", [imgs, h, w], BF16, kind="ExternalOutput")

    # load groups: block 0 and nb-1 single (need memzeroed halo rows); the
    # interior 1..nb-2 in chunks of gT. store groups: 0..nb-2 in chunks of
    # gT (uniform 124-row blocks), nb-1 single (ragged 64-row tail).
    ld_group = {}  # b -> (b0, T) for interior merged loads
    for b0 in range(1, nb - 1, gT):
        T = min(gT, nb - 1 - b0)
        for b in range(b0, b0 + T):
            ld_group[b] = (b0, T)
    st_group = {}  # b -> (b0, T)
    for b0 in range(0, nb - 1, gT):
        T = min(gT, nb - 1 - b0)
        for b in range(b0, b0 + T):
            st_group[b] = (b0, T)

    with tile.TileContext(nc) as tc:
        with (
            tc.tile_pool(name="const", bufs=1) as cpool,
            tc.tile_pool(name="xg", bufs=xgbufs) as xgpool,
            tc.tile_pool(name="xs", bufs=xsbufs) as xspool,
            tc.tile_pool(name="zg", bufs=zgbufs) as zgpool,
            tc.tile_pool(name="zs", bufs=zsbufs) as zspool,
            tc.tile_pool(name="pzp", bufs=pzbufs, space="PSUM") as pzpool,
        ):
            c_t = cpool.tile([128, 15 * ms], BF16)
            d_t = cpool.tile([128, 6 * ms], BF16)
            nc.sync.dma_start(out=c_t[:], in_=c_d[:])
            nc.sync.dma_start(out=d_t[:], in_=d_d[:])

            rings = [nc.sync, nc.scalar]
            ring_cnt = [0]

            def ring():
                e = rings[ring_cnt[0] & 1]
                ring_cnt[0] += 1
                return e

            def body():
                for g in range(imgs):
                    cur_x = None  # (b0, tile)
                    cur_z = None  # (b0, T, tile)
                    for b in range(nb):
                        r0 = b * s
                        # ---- x load ----
                        if b in ld_group:
                            b0, T = ld_group[b]
                            if b == b0:
                                xg = xgpool.tile([128, T, xw], BF16, tag="xg")
                                lo0 = b0 * s - 2
                                base = x_d[0, 0:1, 0:1]
                                src = bass.AP(
                                    tensor=base.tensor,
                                    offset=(g * h + lo0) * w,
                                    ap=[[w, 128], [s * w, T], [1, w]],
                                )
                                ring().dma_start(out=xg[:, :, 2 : 2 + w], in_=src)
                                nc.vector.memzero(xg[:, :, 0:2])
                                nc.vector.memzero(xg[:, :, 2 + w : 4 + w])
                                cur_x = (b0, xg)
                            b0x, xg = cur_x
                            x_t = xg[:, b - b0x, :]
                            k = 128
                        else:
                            lo, hi = max(r0 - 2, 0), min(r0 + s + 2, h)
                            p0, rows = lo - (r0 - 2), hi - lo
                            x_t = xspool.tile([128, xw], BF16, tag="xs")
                            if b == nb - 1:
                                zp0 = max(p for p in (0, 32, 64, 96) if p <= rows)
                                for q0, q1 in ((0, 32), (32, 64), (64, 128)):
                                    if q1 > zp0:
                                        nc.vector.memzero(x_t[max(q0, zp0) : q1, :])
                            ring().dma_start(
                                out=x_t[p0 : p0 + rows, 2 : 2 + w], in_=x_d[g, lo:hi, :]
                            )
                            if b == 0:
                                nc.vector.memzero(x_t[0:2, :])
                            nc.vector.memzero(x_t[:, 0:2])
                            nc.vector.memzero(x_t[:, 2 + w : 4 + w])
                            k = k_last if b == nb - 1 else 128
                        v = 0 if b == 0 else (2 if b == nb - 1 else 1)

                        # ---- z tile ----
                        if b in st_group:
                            b0z, Tz = st_group[b]
                            if b == b0z:
                                cur_z = (b0z, Tz, zgpool.tile([m2, Tz, w], BF16, tag="zg", name="zg"))
                            _, _, zg = cur_z
                            z_view = zg[:, b - b0z, :]
                        else:
                            z_view = zspool.tile([m2, w], BF16, tag="zs")

                        # ---- matmuls + copies ----
                        def copy_out(j, pz):
                            if j % 2 == 0:
                                nc.scalar.copy(
                                    out=z_view[:, nw * j : nw * j + nw], in_=pz[0:m2, :]
                                )
                            else:
                                nc.vector.tensor_copy(
                                    out=z_view[:, nw * j : nw * j + nw], in_=pz[0:m2, :]
                                )

                        if not dxouter:
                            for j in range(nch):
                                pz = pzpool.tile([ms, nw], F32, tag="pz")
                                corrj = (j == 0) or (j == nch - 1)
                                for dx in range(5):
                                    nc.tensor.matmul(
                                        pz[:],
                                        c_t[0:k, (v * 5 + dx) * ms : (v * 5 + dx + 1) * ms],
                                        x_t[0:k, nw * j + dx : nw * j + dx + nw],
                                        start=(dx == 0),
                                        stop=(dx == 4 and not corrj),
                                    )
                                if corrj and j == 0:
                                    nc.tensor.matmul(
                                        pz[:, 0:2],
                                        d_t[0:k, (v * 2 + 0) * ms : (v * 2 + 1) * ms],
                                        x_t[0:k, 2:0:-1],
                                        start=False,
                                        stop=(j != nch - 1),
                                    )
                                if corrj and j == nch - 1:
                                    nc.tensor.matmul(
                                        pz[:, nw - 2 : nw],
                                        d_t[0:k, (v * 2 + 1) * ms : (v * 2 + 2) * ms],
                                        x_t[0:k, w + 2 : w : -1],
                                        start=False,
                                        stop=True,
                                    )
                                copy_out(j, pz)
                        else:
                            pzs = [pzpool.tile([ms, nw], F32, tag="pz", name=f"pz{j}") for j in range(nch)]
                            for dx in range(5):
                                for j in range(nch):
                                    nc.tensor.matmul(
                                        pzs[j][:],
                                        c_t[0:k, (v * 5 + dx) * ms : (v * 5 + dx + 1) * ms],
                                        x_t[0:k, nw * j + dx : nw * j + dx + nw],
                                        start=(dx == 0),
                                        stop=(dx == 4 and j not in (0, nch - 1)),
                                    )
                            nc.tensor.matmul(
                                pzs[0][:, 0:2],
                                d_t[0:k, (v * 2 + 0) * ms : (v * 2 + 1) * ms],
                                x_t[0:k, 2:0:-1],
                                start=False,
                                stop=True,
                            )
                            nc.tensor.matmul(
                                pzs[nch - 1][:, nw - 2 : nw],
                                d_t[0:k, (v * 2 + 1) * ms : (v * 2 + 2) * ms],
                                x_t[0:k, w + 2 : w : -1],
                                start=False,
                                stop=True,
                            )
                            for j in range(nch):
                                copy_out(j, pzs[j])

                        # ---- z store ----
                        if b in st_group:
                            b0z, Tz, zg = cur_z
                            if b == b0z + Tz - 1:
                                zbase = z_d[0, 0:1, 0:1]
                                dst = bass.AP(
                                    tensor=zbase.tensor,
                                    offset=(g * h + b0z * s) * w,
                                    ap=[[w, m2], [s * w, Tz], [1, w]],
                                )
                                ring().dma_start(out=dst, in_=zg[:, :, :])
                        else:
                            rows_out = min(s, h - r0)
                            ring().dma_start(
                                out=z_d[g, r0 : r0 + rows_out, :],
                                in_=z_view[0:rows_out, :],
                            )

            if hwloop > 1:
                with tc.For_i(0, hwloop):
                    body()
            else:
                body()

    nc.compile()
    return nc


def build_nc10(imgs, h, w, nw=512, s=124, hwloop=1, xbufs=8, zbufs=6, pzbufs=8):
    """build_nc8 (per-block DMAs, good DRAM locality) with each block's x
    load and z store alternating between the SP and ACT HW DGE rings, so
    the ~4us per-store completion cost is paid on two rings in parallel."""
    assert w % nw == 0 and nw <= 512
    nb = -(-h // s)
    m2 = s
    ms = 128
    nch = w // nw
    xw = w + 4
    r0_last = (nb - 1) * s
    rows_last = h - (r0_last - 2)
    k_last = rows_last + 2

    nc = bacc.Bacc("TRN2", target_bir_lowering=False, debug=False)
    x_d = nc.dram_tensor("x", [imgs, h, w], BF16, kind="ExternalInput")
    c_d = nc.dram_tensor("b1", [128, 15 * ms], BF16, kind="ExternalInput")
    d_d = nc.dram_tensor("b2", [128, 6 * ms], BF16, kind="ExternalInput")
    z_d = nc.dram_tensor("z", [imgs, h, w], BF16, kind="ExternalOutput")

    blocks = [(g, b) for g in range(imgs) for b in range(nb)]

    with tile.TileContext(nc) as tc:
        with (
            tc.tile_pool(name="const", bufs=1) as cpool,
            tc.tile_pool(name="xp", bufs=xbufs) as xpool,
            tc.tile_pool(name="zp", bufs=zbufs) as zpool,
            tc.tile_pool(name="pzp", bufs=pzbufs, space="PSUM") as pzpool,
        ):
            c_t = cpool.tile([128, 15 * ms], BF16)
            d_t = cpool.tile([128, 6 * ms], BF16)
            nc.sync.dma_start(out=c_t[:], in_=c_d[:])
            nc.sync.dma_start(out=d_t[:], in_=d_d[:])

            def body():
                for t, (g, b) in enumerate(blocks):
                    r0 = b * s
                    lo, hi = max(r0 - 2, 0), min(r0 + s + 2, h)
                    p0, rows = lo - (r0 - 2), hi - lo
                    x_t = xpool.tile([128, xw], BF16, tag="x")
                    if b == nb - 1:
                        zp0 = max(p for p in (0, 32, 64, 96) if p <= rows)
                        for q0, q1 in ((0, 32), (32, 64), (64, 128)):
                            if q1 > zp0:
                                nc.vector.memzero(x_t[max(q0, zp0) : q1, :])
                    ld_eng = nc.sync if t % 2 == 0 else nc.scalar
                    st_eng = nc.scalar if t % 2 == 0 else nc.sync
                    ld_eng.dma_start(
                        out=x_t[p0 : p0 + rows, 2 : 2 + w], in_=x_d[g, lo:hi, :]
                    )
                    if b == 0:
                        nc.vector.memzero(x_t[0:2, :])
                    nc.vector.memzero(x_t[:, 0:2])
                    nc.vector.memzero(x_t[:, 2 + w : 4 + w])
                    k = k_last if b == nb - 1 else 128
                    v = 0 if b == 0 else (2 if b == nb - 1 else 1)
                    rows_out = min(s, h - r0)
                    z_t = zpool.tile([m2, w], BF16, tag="z")
                    for j in range(nch):
                        pz = pzpool.tile([ms, nw], F32, tag="pz")
                        corrj = (j == 0) or (j == nch - 1)
                        for dx in range(5):
                            nc.tensor.matmul(
                                pz[:],
                                c_t[0:k, (v * 5 + dx) * ms : (v * 5 + dx + 1) * ms],
                                x_t[0:k, nw * j + dx : nw * j + dx + nw],
                                start=(dx == 0),
                                stop=(dx == 4 and not corrj),
                            )
                        if corrj and j == 0:
                            nc.tensor.matmul(
                                pz[:, 0:2],
                                d_t[0:k, (v * 2 + 0) * ms : (v * 2 + 1) * ms],
                                x_t[0:k, 2:0:-1],
                                start=False,
                                stop=(j != nch - 1),
                            )
                        if corrj and j == nch - 1:
                            nc.tensor.matmul(
                                pz[:, nw - 2 : nw],
                                d_t[0:k, (v * 2 + 1) * ms : (v * 2 + 2) * ms],
                                x_t[0:k, w + 2 : w : -1],
                                start=False,
                                stop=True,
                            )
                        if j % 2 == 0:
                            nc.scalar.copy(
                                out=z_t[:, nw * j : nw * j + nw], in_=pz[0:m2, :]
                            )
                        else:
                            nc.vector.tensor_copy(
                                out=z_t[:, nw * j : nw * j + nw], in_=pz[0:m2, :]
                            )
                    st_eng.dma_start(
                        out=z_d[g, r0 : r0 + rows_out, :], in_=z_t[0:rows_out, :]
                    )

            if hwloop > 1:
                with tc.For_i(0, hwloop):
                    body()
            else:
                body()

    nc.compile()
    return nc


def build_nc11(
    imgs, h, w, nw=512, s=124, hwloop=1, xbufs=8, zbufs=6, pzbufs=8, gp_every=3
):
    """build_nc10 + every gp_every-th z store issued via the gpsimd SWDGE,
    adding a third independent DMA path (~37 GB/s) to the ~93 GB/s HWDGE
    store cap that paces the whole kernel."""
    assert w % nw == 0 and nw <= 512
    nb = -(-h // s)
    m2 = s
    ms = 128
    nch = w // nw
    xw = w + 4
    r0_last = (nb - 1) * s
    rows_last = h - (r0_last - 2)
    k_last = rows_last + 2

    nc = bacc.Bacc("TRN2", target_bir_lowering=False, debug=False)
    x_d = nc.dram_tensor("x", [imgs, h, w], BF16, kind="ExternalInput")
    c_d = nc.dram_tensor("b1", [128, 15 * ms], BF16, kind="ExternalInput")
    d_d = nc.dram_tensor("b2", [128, 6 * ms], BF16, kind="ExternalInput")
    z_d = nc.dram_tensor("z", [imgs, h, w], BF16, kind="ExternalOutput")

    blocks = [(g, b) for g in range(imgs) for b in range(nb)]

    with tile.TileContext(nc) as tc:
        with (
            tc.tile_pool(name="const", bufs=1) as cpool,
            tc.tile_pool(name="xp", bufs=xbufs) as xpool,
            tc.tile_pool(name="zp", bufs=zbufs) as zpool,
            tc.tile_pool(name="pzp", bufs=pzbufs, space="PSUM") as pzpool,
        ):
            c_t = cpool.tile([128, 15 * ms], BF16)
            d_t = cpool.tile([128, 6 * ms], BF16)
            nc.sync.dma_start(out=c_t[:], in_=c_d[:])
            nc.sync.dma_start(out=d_t[:], in_=d_d[:])

            def body():
                for t, (g, b) in enumerate(blocks):
                    r0 = b * s
                    lo, hi = max(r0 - 2, 0), min(r0 + s + 2, h)
                    p0, rows = lo - (r0 - 2), hi - lo
                    x_t = xpool.tile([128, xw], BF16, tag="x")
                    if b == nb - 1:
                        zp0 = max(p for p in (0, 32, 64, 96) if p <= rows)
                        for q0, q1 in ((0, 32), (32, 64), (64, 128)):
                            if q1 > zp0:
                                nc.vector.memzero(x_t[max(q0, zp0) : q1, :])
                    ld_eng = nc.sync if t % 2 == 0 else nc.scalar
                    ld_eng.dma_start(
                        out=x_t[p0 : p0 + rows, 2 : 2 + w], in_=x_d[g, lo:hi, :]
                    )
                    if b == 0:
                        nc.vector.memzero(x_t[0:2, :])
                    nc.vector.memzero(x_t[:, 0:2])
                    nc.vector.memzero(x_t[:, 2 + w : 4 + w])
                    k = k_last if b == nb - 1 else 128
                    v = 0 if b == 0 else (2 if b == nb - 1 else 1)
                    rows_out = min(s, h - r0)
                    z_t = zpool.tile([m2, w], BF16, tag="z")
                    for j in range(nch):
                        pz = pzpool.tile([ms, nw], F32, tag="pz")
                        corrj = (j == 0) or (j == nch - 1)
                        for dx in range(5):
                            nc.tensor.matmul(
                                pz[:],
                                c_t[0:k, (v * 5 + dx) * ms : (v * 5 + dx + 1) * ms],
                                x_t[0:k, nw * j + dx : nw * j + dx + nw],
                                start=(dx == 0),
                                stop=(dx == 4 and not corrj),
                            )
                        if corrj and j == 0:
                            nc.tensor.matmul(
                                pz[:, 0:2],
                                d_t[0:k, (v * 2 + 0) * ms : (v * 2 + 1) * ms],
                                x_t[0:k, 2:0:-1],
                                start=False,
                                stop=(j != nch - 1),
                            )
                        if corrj and j == nch - 1:
                            nc.tensor.matmul(
                                pz[:, nw - 2 : nw],
                                d_t[0:k, (v * 2 + 1) * ms : (v * 2 + 2) * ms],
                                x_t[0:k, w + 2 : w : -1],
                                start=False,
                                stop=True,
                            )
                        if copies == "act" or j % 2 == 0:
                            nc.scalar.copy(
                                out=z_t[:, nw * j : nw * j + nw], in_=pz[0:m2, :]
                            )
                        else:
                            nc.vector.tensor_copy(
                                out=z_t[:, nw * j : nw * j + nw], in_=pz[0:m2, :]
                            )
                    if gp_every and t % gp_every == gp_every - 1:
                        st_eng = nc.gpsimd
                    else:
                        st_eng = nc.scalar if t % 2 == 0 else nc.sync
                    st_eng.dma_start(
                        out=z_d[g, r0 : r0 + rows_out, :], in_=z_t[0:rows_out, :]
                    )

            if hwloop > 1:
                with tc.For_i(0, hwloop):
                    body()
            else:
                body()

    nc.compile()
    return nc


def _shift_bands_v0(b1, b2, m2):
    """Shift the top-block (variant 0) band sections up 2 k-rows so block 0
    can load x rows 0..126 at partition 0 with no zero-row padding: the
    k=0,1 taps (x rows -2,-1) multiplied zeros before; after the shift they
    simply don't exist."""
    c = b1.reshape(128, 3, 5, m2).copy()
    d = b2.reshape(128, 3, 2, m2).copy()
    c[:-2, 0] = c[2:, 0]
    c[-2:, 0] = 0.0
    d[:-2, 0] = d[2:, 0]
    d[-2:, 0] = 0.0
    return (
        np.ascontiguousarray(c.reshape(128, 15 * m2)),
        np.ascontiguousarray(d.reshape(128, 6 * m2)),
    )


def build_nc12(
    imgs,
    h,
    w,
    nw=512,
    s=124,
    hwloop=1,
    xbufs=10,
    zbufs=8,
    pzbufs=8,
    gp_every=3,
    bodies_per_iter=1,
    copies="split",
):
    """build_nc11 with every memset eliminated from the block loop:

    - bands from _shift_bands_v0: block 0 loads x rows 0..s+2 at partition
      0 (k=126), no zero-row padding;
    - bottom block uses k=rows actually loaded (66) — the virtual zero rows
      below the image are simply not contracted;
    - the 4 halo columns of each x pool buffer are zeroed ONCE before the
      loop (pool rotation is static), never touched by the data DMAs, and
      read as zeros forever after.

    This removes all per-block DVE memsets and their DMA->memset->matmul
    dependency chains, and reduces DVE port pressure (which stalls gpsimd
    SWDGE descriptor generation).
    """
    assert w % nw == 0 and nw <= 512
    nb = -(-h // s)
    m2 = s
    ms = 128
    nch = w // nw
    xw = w + 4

    nc = bacc.Bacc("TRN2", target_bir_lowering=False, debug=False)
    x_d = nc.dram_tensor("x", [imgs, h, w], BF16, kind="ExternalInput")
    c_d = nc.dram_tensor("b1", [128, 15 * ms], BF16, kind="ExternalInput")
    d_d = nc.dram_tensor("b2", [128, 6 * ms], BF16, kind="ExternalInput")
    z_d = nc.dram_tensor("z", [imgs, h, w], BF16, kind="ExternalOutput")

    blocks = [(g, b) for g in range(imgs) for b in range(nb)]

    with tile.TileContext(nc) as tc:
        with (
            tc.tile_pool(name="const", bufs=1) as cpool,
            tc.tile_pool(name="xp", bufs=xbufs) as xpool,
            tc.tile_pool(name="zp", bufs=zbufs) as zpool,
            tc.tile_pool(name="pzp", bufs=pzbufs, space="PSUM") as pzpool,
        ):
            c_t = cpool.tile([128, 15 * ms], BF16)
            d_t = cpool.tile([128, 6 * ms], BF16)
            nc.sync.dma_start(out=c_t[:], in_=c_d[:])
            nc.sync.dma_start(out=d_t[:], in_=d_d[:])

            def body():
                for t, (g, b) in enumerate(blocks):
                    r0 = b * s
                    lo, hi = max(r0 - 2, 0), min(r0 + s + 2, h)
                    rows = hi - lo
                    k = rows
                    x_t = xpool.tile([128, xw], BF16, tag="x")
                    ld_eng = nc.sync if t % 2 == 0 else nc.scalar
                    ld_eng.dma_start(out=x_t[0:rows, 2 : 2 + w], in_=x_d[g, lo:hi, :])
                    nc.vector.memzero(x_t[:, 0:2])
                    nc.vector.memzero(x_t[:, 2 + w : 4 + w])
                    v = 0 if b == 0 else (2 if b == nb - 1 else 1)
                    rows_out = min(s, h - r0)
                    z_t = zpool.tile([m2, w], BF16, tag="z")
                    for j in range(nch):
                        pz = pzpool.tile([ms, nw], F32, tag="pz")
                        corrj = (j == 0) or (j == nch - 1)
                        for dx in range(5):
                            nc.tensor.matmul(
                                pz[:],
                                c_t[0:k, (v * 5 + dx) * ms : (v * 5 + dx + 1) * ms],
                                x_t[0:k, nw * j + dx : nw * j + dx + nw],
                                start=(dx == 0),
                                stop=(dx == 4 and not corrj),
                            )
                        if corrj and j == 0:
                            nc.tensor.matmul(
                                pz[:, 0:2],
                                d_t[0:k, (v * 2 + 0) * ms : (v * 2 + 1) * ms],
                                x_t[0:k, 2:0:-1],
                                start=False,
                                stop=(j != nch - 1),
                            )
                        if corrj and j == nch - 1:
                            nc.tensor.matmul(
                                pz[:, nw - 2 : nw],
                                d_t[0:k, (v * 2 + 1) * ms : (v * 2 + 2) * ms],
                                x_t[0:k, w + 2 : w : -1],
                                start=False,
                                stop=True,
                            )
                        if copies == "act" or j % 2 == 0:
                            nc.scalar.copy(
                                out=z_t[:, nw * j : nw * j + nw], in_=pz[0:m2, :]
                            )
                        else:
                            nc.vector.tensor_copy(
                                out=z_t[:, nw * j : nw * j + nw], in_=pz[0:m2, :]
                            )
                    if gp_every == 7 and t % 7 in (2, 5):
                        st_eng = nc.gpsimd
                    elif gp_every and gp_every != 7 and t % gp_every == gp_every - 1:
                        st_eng = nc.gpsimd
                    else:
                        st_eng = nc.scalar if t % 2 == 0 else nc.sync
                    st_eng.dma_start(
                        out=z_d[g, r0 : r0 + rows_out, :], in_=z_t[0:rows_out, :]
                    )

            if hwloop > 1:
                with tc.For_i(0, hwloop):
                    for _ in range(bodies_per_iter):
                        body()
            else:
                body()

    nc.compile()
    return nc


def _pad_bands_128(b1, b2, m2):
    c = b1.reshape(128, 15, m2)
    cp = np.zeros((128, 15, 128), b1.dtype)
    cp[:, :, :m2] = c
    d = b2.reshape(128, 6, m2)
    dp = np.zeros((128, 6, 128), b2.dtype)
    dp[:, :, :m2] = d
    return (
        np.ascontiguousarray(cp.reshape(128, 15 * 128)),
        np.ascontiguousarray(dp.reshape(128, 6 * 128)),
    )


def to_bf16(a):
    import ml_dtypes

    return np.ascontiguousarray(np.asarray(a).astype(ml_dtypes.bfloat16))


def make_in_maps(x, w1, w2, h=FULL_H, s=124):
    """bf16 per-core input maps (v0-shifted + 128-padded bands) from full
    f32 inputs; matches build_nc12."""
    nb = -(-h // s)
    b1, b2 = _build_bands5(w1, w2, h, s, nb)
    b1, b2 = _shift_bands_v0(b1, b2, s)
    b1, b2 = _pad_bands_128(b1, b2, s)
    b1, b2 = to_bf16(b1), to_bf16(b2)
    xb = to_bf16(np.asarray(x, np.float32).reshape(FULL_B, FULL_H, FULL_W))
    imgs = FULL_B // NCORES
    return [
        {"x": np.ascontiguousarray(xb[imgs * c : imgs * (c + 1)]), "b1": b1, "b2": b2}
        for c in range(NCORES)
    ]


_NC_CACHE = None


def kernel(x, w1, w2):
    global _NC_CACHE, LAST_RESULTS
    in_maps = make_in_maps(x, w1, w2)
    if _NC_CACHE is None:
        _NC_CACHE = build_nc12(FULL_B // NCORES, FULL_H, FULL_W, nw=512, s=124)
    nc = _NC_CACHE
    res = run_bass_kernel_spmd(nc, in_maps, core_ids=list(range(NCORES)), trace=TRACE)
    LAST_RESULTS = res
    out = np.stack(
        [np.asarray(res.results[c]["z"], np.float32) for c in range(NCORES)], axis=0
    )
    return out.reshape(FULL_B, 1, FULL_H, FULL_W)

